# revision 24
# baseline (speedup 1.0000x reference)
"""Trainium2 Bass kernel for nn_C3DNet — data-parallel over the 10 samples on 8 cores.

Math (per sample, from the reference):
  x:(52,7,24) -conv1(6,2,2)s(2,1,2)+sig-> (24,6,12) -conv2(4,1,2)s(4,1,2)+sig-> (6,6,6)
  -avgpool2-> 27 -fc4+sig-> 80 -fc5+sig-> 200 -fc6+sig-> 676
  out = h6.reshape(13,52) @ x.reshape(52,168)  -> (13,168) -> 2184

Everything is cast as TensorE matmuls (bf16 datapath, f32 PSUM):
  * conv1/conv2 contract the D dimension (on partitions) using host-built
    banded weight matrices; the (h,w) taps become strided free-dim views.
  * fc4 contracts q=3 partitions x 9 (hp,wp) matmuls; b1/b2/b4 applied via the
    ACT sigmoid's per-partition bias operand; b5/b6 folded via ones-rows.
  * fc6 emits PSUM [52, (i,s)] directly so the final einsum lhsT needs no
    transpose; its 26 matmuls are split into a 13-a (k-chunk 0) + 13-b
    (k-chunk 1) sequence gated by per-half fc5 sigmoids so the a-half starts
    one sigmoid earlier.

Schedule notes (v2, from the v1 perfetto trace):
  * w6 (286KB bf16) was the body bottleneck in v1: whole-row-contiguous DMAs
    were each pinned to a single DMA engine (~22GB/s), landing at ~16us and
    gating fc6. v2 ships w6 as four 338-column chunks (676B rows, the same
    row size as x, which provably round-robins across all 16 DMA engines)
    spread over the SP and DVE HWDGE rings, all triggered in the preamble.
  * The Activation ring issues NO DMAs: ACT does the sigmoid-table preload
    dummy immediately (gated only on the DVE zb/psum_scr memsets), then is
    free for the sigmoid chain.
  * Pool memsets touch ONLY the ones-rows (disjoint from the sigmoid output
    rows), removing the write-order hazard; PE waits ssem once before conv2.
  * Output: einsum region s is copied PSUM->SBUF by ACT (s=0) / DVE (s=1) in
    parallel, and each region gets its own DRAM store (SP ring / DVE ring).

Raw-bass (Block + explicit semaphores): this walrus build only supports ONE
attached sync-wait per Matmult/DMA instruction, so standalone wait_ge
instructions are used. DMA completion order is not guaranteed across queues,
so consumers wait for the issuing group's FULL credit count (16 per DMA).
"""

import sys
from contextlib import ExitStack

sys.path.insert(0, "/opt/trn_rl_repo")

import numpy as np
import ml_dtypes

# Each DMA delivers 16 completion credits; waiting below 16 proved
# nondeterministic on this runtime, so all consumers wait for the full count.
_DMA_CREDITS = 16

BF16 = ml_dtypes.bfloat16

N_CORES = 8
NS = 2  # sample slots per core
# core i handles samples ASSIGN[i]; host gathers accordingly
ASSIGN = [[0, 8], [1, 9]] + [[i, i] for i in range(2, N_CORES)]

LAST_EXEC_NS = None
LAST_RESULT = None

_BUILT = {}


def _build_nc():
    import concourse.bass as bass
    import concourse.mybir as mybir

    f32 = mybir.dt.float32
    bf16 = mybir.dt.bfloat16
    Sig = mybir.ActivationFunctionType.Sigmoid

    nc = bass.Bass()

    # x rows 0:52 = sample data, row 52 = ones (carries b1 via wb row 52)
    x_d = nc.declare_dram_parameter("x", [64, NS * 168], bf16, isOutput=False)
    # wb: w1b (96 cols, rows 0:53 incl. b1 ones-row) ++ w2b (12, rows 0:25
    # incl. b2 ones-row)
    wb_d = nc.declare_dram_parameter("wb", [64, 108], bf16, isOutput=False)
    # w4p row 6 = b4 in the j=0 block, zeros elsewhere
    w4p_d = nc.declare_dram_parameter("w4p", [12, 720], bf16, isOutput=False)
    w5t_d = nc.declare_dram_parameter("w5t", [86, 200], bf16, isOutput=False)
    # w6 packed: cols 0:676 = k-chunk a (rows 0:100 = w6[:, :100].T),
    # cols 676:1352 = k-chunk b (rows 0:100 = w6[:, 100:200].T, row 100 = b6)
    w6_d = nc.declare_dram_parameter("w6", [106, 1352], bf16, isOutput=False)
    out_d = nc.declare_dram_parameter("out", [13, NS * 168], f32, isOutput=True)

    es = ExitStack()

    def sb(name, shape, dt=bf16):
        return es.enter_context(nc.sbuf_tensor(name, shape, dt))

    def pt(name, shape):
        return es.enter_context(nc.psum_tensor(name, shape, f32))

    with es:
        x_t = sb("x_t", [64, NS * 168])
        wb_t = sb("wb_t", [64, 108])
        w4p_t = sb("w4p_t", [12, 720])
        w5t_t = sb("w5t_t", [86, 200])
        w6_t = sb("w6_t", [106, 1352])
        h1_t = sb("h1_t", [25, NS * 72])   # row 24 = ones (b2 rides w2b row 24)
        h2_t = sb("h2_t", [6, NS * 36])
        tmp6_t = sb("tmp6_t", [6, NS * 18])
        pool_t = sb("pool_t", [7, NS * 9])  # row 6 = ones (b4 rides w4p row 6)
        h4_t = sb("h4_t", [81, NS])         # row 80 = ones (b5 rides w5t row 80)
        t01 = sb("t01", [101, 2 * NS])      # cols 0:2 = t0, 2:4 = t1; row 100 = ones
        h6_t = sb("h6_t", [52, 13 * NS])
        out_t = sb("out_t", [13, NS * 168], f32)
        scr_t = sb("scr_t", [1, 2])         # bf16: table-preload dummy output
        zb_t = sb("zb_t", [101, 1], f32)    # zero bias for all sigmoids

        psum1 = pt("psum1", [24, NS * 72])
        psum2 = pt("psum2", [6, NS * 36])
        psum4 = pt("psum4", [80, NS])
        psum5 = pt("psum5", [100, 2 * NS])
        psum6 = pt("psum6", [52, 13 * NS])
        psume = pt("psume", [13, NS * 168])
        psum_scr = pt("psum_scr", [1, 2])

        dsA = es.enter_context(nc.semaphore("dsA"))    # wb (SP) + x (ACT)
        dsE = es.enter_context(nc.semaphore("dsE"))    # w4p (SWDGE)
        dsF = es.enter_context(nc.semaphore("dsF"))    # w5t (SWDGE)
        dsG = es.enter_context(nc.semaphore("dsG"))    # w6 col-chunks x6 (SP+ACT)
        dsO = es.enter_context(nc.semaphore("dsO"))    # output stores (no waiter)
        ssem = es.enter_context(nc.semaphore("ssem"))  # Pool ones-row memsets
        ssev = es.enter_context(nc.semaphore("ssev"))  # DVE zb/psum_scr memsets
        psem = es.enter_context(nc.semaphore("psem"))
        asem = es.enter_context(nc.semaphore("asem"))
        vsem = es.enter_context(nc.semaphore("vsem"))

        with nc.Block() as block:
            hoist = nc._hoist_insts = []

            @block.sync
            def _(sync):
                # wb first (tiny, conv1's stationary), then four w6 column
                # chunks. DMAs over ~64KB get pinned to 1-2 DMA engines;
                # <=48KB chunks spread by-row across all 16.
                hoist.append(sync.dma_start(out=wb_t[:], in_=wb_d[:]).then_inc(dsA, 16))
                hoist.append(
                    sync.dma_start(out=w6_t[:, 0:226], in_=w6_d[:, 0:226]).then_inc(dsG, 16)
                )
                hoist.append(
                    sync.dma_start(out=w6_t[:, 226:452], in_=w6_d[:, 226:452]).then_inc(dsG, 16)
                )
                hoist.append(
                    sync.dma_start(out=w6_t[:, 452:676], in_=w6_d[:, 452:676]).then_inc(dsG, 16)
                )
                hoist.append(
                    sync.dma_start(out=w6_t[:, 676:902], in_=w6_d[:, 676:902]).then_inc(dsG, 16)
                )
                # output region 0 once ACT's first copy lands
                sync.wait_ge(asem, 7)
                sync.dma_start(out=out_d[:, 0:168], in_=out_t[:, 0:168]).then_inc(dsO, 16)

            @block.vector
            def _(vector):
                # zb + psum_scr memsets gate only the ACT table preload
                hoist.append(vector.memset(psum_scr[:], 0.0).then_inc(ssev))
                hoist.append(vector.memset(zb_t[:], 0.0).then_inc(ssev))
                # (h, w) pooling as ONE 4-tap reduce over the (dh, dw) dims
                vector.wait_ge(ssem, 4)
                vector.wait_ge(asem, 2)
                h2r = h2_t[:].rearrange(
                    "p (s hp dh wp dw) -> p (s hp) wp dh dw", s=NS, hp=3, dh=2, wp=3, dw=2
                )
                poolr = pool_t[0:6, :].rearrange("p (s hp wp) -> p (s hp) wp", s=NS, hp=3, wp=3)
                with nc.allow_low_precision("4-term bf16 pooling sum, matches prior impl"):
                    vector.tensor_reduce(
                        poolr[:], h2r[:], axis=mybir.AxisListType.XY, op=mybir.AluOpType.add
                    ).then_inc(vsem)  # 1


            @block.gpsimd
            def _(gpsimd):
                # weight DMAs first (SWDGE gen is ~870ns each), then the
                # ones-row memsets (disjoint from all sigmoid output rows)
                hoist.append(gpsimd.dma_start(out=w4p_t[:], in_=w4p_d[:]).then_inc(dsE, 16))
                hoist.append(gpsimd.dma_start(out=w5t_t[:], in_=w5t_d[:]).then_inc(dsF, 16))
                # whole-tensor ones memsets (APs must start at partition 0);
                # writers of the non-ones rows wait on ssem first
                hoist.append(gpsimd.memset(h1_t[:], 1.0).then_inc(ssem))
                hoist.append(gpsimd.memset(pool_t[:], 1.0).then_inc(ssem))
                hoist.append(gpsimd.memset(h4_t[:], 1.0).then_inc(ssem))
                hoist.append(gpsimd.memset(t01[:], 1.0).then_inc(ssem))

            @block.scalar
            def _(scalar):
                # x rides the ACT ring first-thing: the ACT engine starts
                # ~260ns before SP, so conv1's gate lands earliest here
                hoist.append(scalar.dma_start(out=x_t[:], in_=x_d[:]).then_inc(dsA, 16))
                scalar.wait_ge(ssev, 2)
                # dummy sigmoid FIRST IN THIS BASIC BLOCK: walrus tracks ACT
                # tables per-bb, so the preload must live in the same bb as
                # the real sigmoids to avoid a 1.3us reload before sig1
                scalar.activation(scr_t[:], psum_scr[:], Sig, bias=zb_t[0:1, :])
                # last two w6 chunks after the table load, before sig1 is needed
                scalar.dma_start(out=w6_t[:, 902:1128], in_=w6_d[:, 902:1128]).then_inc(dsG, 16)
                scalar.dma_start(out=w6_t[:, 1128:1352], in_=w6_d[:, 1128:1352]).then_inc(dsG, 16)
                scalar.wait_ge(ssem, 4)
                scalar.wait_ge(psem, 1)
                scalar.activation(h1_t[0:24, :], psum1[:], Sig, bias=zb_t[0:24, :]).then_inc(asem)  # 1
                scalar.wait_ge(psem, 2)
                scalar.activation(h2_t[:], psum2[:], Sig, bias=zb_t[0:6, :]).then_inc(asem)  # 2
                scalar.wait_ge(psem, 3)
                scalar.activation(h4_t[0:80, :], psum4[:], Sig, bias=zb_t[0:80, :]).then_inc(asem)  # 3
                scalar.wait_ge(psem, 4)
                scalar.activation(
                    t01[0:100, 0:NS], psum5[:, 0:NS], Sig, bias=zb_t[0:100, :]
                ).then_inc(asem)  # 4
                scalar.wait_ge(psem, 5)
                scalar.activation(
                    t01[0:100, NS : 2 * NS], psum5[:, NS : 2 * NS], Sig, bias=zb_t[0:100, :]
                ).then_inc(asem)  # 5
                scalar.wait_ge(psem, 6)
                scalar.activation(h6_t[:], psum6[:], Sig, bias=zb_t[0:52, :]).then_inc(asem)  # 6
                scalar.wait_ge(psem, 7)
                scalar.copy(out_t[:, 0:168], psume[:, 0:168]).then_inc(asem)  # 7
                scalar.wait_ge(psem, 8)
                scalar.copy(out_t[:, 168:336], psume[:, 168:336]).then_inc(asem)  # 8
                # region-1 store on ACT's own ring; same-engine ordering still
                # needs the semaphore round-trip (DMA data fetch is async)
                scalar.wait_ge(asem, 8)
                scalar.dma_start(out=out_d[:, 168:336], in_=out_t[:, 168:336]).then_inc(dsO, 16)

            @block.tensor
            def _(tensor):
                # conv1: 4 accumulated matmuls; K=53 incl. the b1 ones-row
                tensor.wait_ge(dsA, 2 * _DMA_CREDITS)  # wb (SP) + x (ACT)
                x4 = x_t[0:53, :].rearrange("p (s h w) -> p s h w", s=NS, h=7, w=24)
                taps1 = [(kh, kw) for kh in range(2) for kw in range(2)]
                for k, (kh, kw) in enumerate(taps1):
                    mm = tensor.matmul(
                        psum1[:],
                        wb_t[0:53, k * 24 : (k + 1) * 24],
                        x4[:, :, kh : kh + 6, kw : kw + 23 : 2],
                        start=(k == 0),
                        stop=(k == 3),
                    )
                    if k == 3:
                        mm.then_inc(psem)  # psem 1
                # conv2: K=25 incl. the b2 ones-row. asem>=1 transitively
                # covers the Pool ones memsets (ACT waits ssem before sig1).
                tensor.wait_ge(asem, 1)
                h14 = h1_t[:].rearrange("p (s h w) -> p s h w", s=NS, h=6, w=12)
                for kw in range(2):
                    mm = tensor.matmul(
                        psum2[:],
                        wb_t[0:25, 96 + kw * 6 : 96 + (kw + 1) * 6],
                        h14[:, :, :, kw : kw + 11 : 2],
                        start=(kw == 0),
                        stop=(kw == 1),
                    )
                    if kw == 1:
                        mm.then_inc(psem)  # psem 2
                # fc4: 9 (hp,wp) matmuls vs the h/w-pooled tile; d-pooling and
                # /8 live in w4p; j=0 has K=7 incl. the b4 ones-row
                tensor.wait_ge(vsem, 1)
                tensor.wait_ge(dsE, 16)
                pool4 = pool_t[:].rearrange("p (s j) -> p s j", s=NS, j=9)
                for j in range(9):
                    kk = 7 if j == 0 else 6
                    mm = tensor.matmul(
                        psum4[:],
                        w4p_t[0:kk, j * 80 : (j + 1) * 80],
                        pool4[0:kk, :, j],
                        start=(j == 0),
                        stop=(j == 8),
                    )
                    if j == 8:
                        mm.then_inc(psem)  # psem 3
                # fc5: two k-halves, each gating its own sigmoid so fc6's
                # a-matmuls can start one ACT op earlier
                tensor.wait_ge(asem, 3)
                tensor.wait_ge(dsF, 16)
                tensor.matmul(
                    psum5[:, 0:NS], w5t_t[0:81, 0:100], h4_t[:], start=True, stop=True
                ).then_inc(psem)  # psem 4
                tensor.matmul(
                    psum5[:, NS : 2 * NS], w5t_t[0:81, 100:200], h4_t[:], start=True, stop=True
                ).then_inc(psem)  # psem 5
                # fc6: 13 i-chunks x 2 k-chunks (a_i, b_i interleaved — only
                # one PSUM accumulation group may be open per zero region)
                tensor.wait_ge(asem, 5)
                tensor.wait_ge(dsG, 96)
                for i in range(13):
                    tensor.matmul(
                        psum6[:, i * NS : (i + 1) * NS],
                        w6_t[0:100, i * 52 : (i + 1) * 52],
                        t01[0:100, 0:NS],
                        start=True,
                        stop=False,
                    )
                    mm = tensor.matmul(
                        psum6[:, i * NS : (i + 1) * NS],
                        w6_t[0:101, 676 + i * 52 : 676 + (i + 1) * 52],
                        t01[:, NS : 2 * NS],
                        start=False,
                        stop=True,
                    )
                    if i == 12:
                        mm.then_inc(psem)  # psem 6
                # einsum; each sample region releases its own copy engine
                tensor.wait_ge(asem, 6)
                h6v = h6_t[:].rearrange("p (i s) -> p s i", s=NS)
                for s in range(NS):
                    tensor.matmul(
                        psume[:, s * 168 : (s + 1) * 168],
                        h6v[:, s, :],
                        x_t[0:52, s * 168 : (s + 1) * 168],
                        start=True,
                        stop=True,
                    ).then_inc(psem)  # psem 7, 8

    _strip_entry_barrier(nc)
    return nc


def _strip_entry_barrier(nc):
    f = nc.m.functions[0]
    bbs = {bb.name: bb for bb in f.blocks}
    main = bbs["main"]
    # 1) drop the init all-engine barrier (nothing reads the const-AP tiles)
    main.instructions = [
        i
        for i in main.instructions
        if not (
            i.name.startswith("barrier_")
            or getattr(i, "opcode", "") == "Drain"
            or type(i).__name__ == "InstDrain"
        )
    ]
    # 2) hoist the input-DMA triggers into main so transfers start during the
    #    preamble, before the Block-entry rendezvous
    hoisted = {bi.ins.name for bi in getattr(nc, "_hoist_insts", [])}
    if hoisted:
        moved = []
        for bb in f.blocks:
            if bb.name == "main" or not bb.instructions:
                continue
            keep = []
            for i in bb.instructions:
                (moved if i.name in hoisted else keep).append(i)
            if len(keep) != len(bb.instructions):
                bb.instructions = keep
        # insert at the very top of main (after the entry Call): the DMA
        # triggers use only immediates + the parameter table, not the
        # preamble registers
        insts = main.instructions
        main.instructions = insts[:1] + moved + insts[1:]


def _prep_weights(w1, b1, w2, b2, w4, b4, w5, b5, w6, b6):
    f = np.float32
    w1v = np.asarray(w1, f)[0, 0]  # (6,2,2)
    w2v = np.asarray(w2, f)[0, 0, :, 0, :]  # (4,2)
    w4 = np.asarray(w4, f)
    w5 = np.asarray(w5, f)
    w6 = np.asarray(w6, f)
    b1 = np.asarray(b1, f)
    b2 = np.asarray(b2, f)
    b4 = np.asarray(b4, f)
    b5 = np.asarray(b5, f)
    b6 = np.asarray(b6, f)

    wb = np.zeros((64, 108), f)
    for kd in range(6):
        for kh in range(2):
            for kw in range(2):
                for d in range(24):
                    wb[2 * d + kd, (kh * 2 + kw) * 24 + d] = w1v[kd, kh, kw]
    wb[52, 0:24] = b1[0]  # ones-row bias, k=0 tap block only
    for kd in range(4):
        for kw in range(2):
            for d in range(6):
                wb[4 * d + kd, 96 + kw * 6 + d] = w2v[kd, kw]
    wb[24, 96:102] = b2[0]  # ones-row bias, kw=0 block only

    w4r = w4.reshape(80, 3, 3, 3) / 8.0
    w4q = np.transpose(w4r, (1, 2, 3, 0)).reshape(3, 720)
    w4p = np.zeros((12, 720), f)
    w4p[0:6:2, :] = w4q
    w4p[1:6:2, :] = w4q
    w4p[6, 0:80] = b4  # ones-row bias, j=0 block only

    w5t = np.zeros((86, 200), f)
    w5t[0:80, :] = w5.T
    w5t[80, :] = b5

    w6p = np.zeros((106, 1352), f)
    w6p[0:100, 0:676] = w6[:, 0:100].T
    w6p[0:100, 676:1352] = w6[:, 100:200].T
    w6p[100, 676:1352] = b6

    return dict(
        wb=wb.astype(BF16),
        w4p=w4p.astype(BF16),
        w5t=w5t.astype(BF16),
        w6=w6p.astype(BF16),
    )


def kernel(x, w1, b1, w2, b2, w4, b4, w5, b5, w6, b6, _trace=False):
    global LAST_EXEC_NS, LAST_RESULT
    from concourse.bass_utils import run_bass_kernel_spmd

    if "nc" not in _BUILT:
        _BUILT["nc"] = _build_nc()
    nc = _BUILT["nc"]

    xs = np.ascontiguousarray(np.asarray(x, np.float32).reshape(10, 52, 168))
    wd = _prep_weights(w1, b1, w2, b2, w4, b4, w5, b5, w6, b6)

    in_maps = []
    for i in range(N_CORES):
        xc = np.ones((64, NS * 168), np.float32)
        xc[0:52] = np.transpose(np.stack([xs[a] for a in ASSIGN[i]]), (1, 0, 2)).reshape(52, NS * 168)
        xc = np.ascontiguousarray(xc.astype(BF16))
        m = {"x": xc}
        m.update(wd)
        in_maps.append(m)

    res = run_bass_kernel_spmd(nc, in_maps, core_ids=list(range(N_CORES)), trace=_trace)
    LAST_EXEC_NS = res.exec_time_ns
    LAST_RESULT = res

    out = np.zeros((10, 2184), np.float32)
    for i in range(N_CORES):
        o = res.results[i]["out"].reshape(13, NS, 168)
        for slot, b in enumerate(ASSIGN[i]):
            out[b] = o[:, slot, :].reshape(2184)
    return out


# revision 26
# speedup vs baseline: 1.1313x; 1.1313x over previous
"""Trainium2 Bass kernel for nn_C3DNet — data-parallel over the 10 samples on 8 cores.

Math (per sample, from the reference):
  x:(52,7,24) -conv1(6,2,2)s(2,1,2)+sig-> (24,6,12) -conv2(4,1,2)s(4,1,2)+sig-> (6,6,6)
  -avgpool2-> 27 -fc4+sig-> 80 -fc5+sig-> 200 -fc6+sig-> 676
  out = h6.reshape(13,52) @ x.reshape(52,168)  -> (13,168) -> 2184

Everything is cast as TensorE matmuls (bf16 datapath, f32 PSUM):
  * conv1/conv2 contract the D dimension (on partitions) using host-built
    banded weight matrices; the (h,w) taps become strided free-dim views.
  * fc4 contracts q=3 partitions x 9 (hp,wp) matmuls; b1/b2/b4 applied via the
    ACT sigmoid's per-partition bias operand; b5/b6 folded via ones-rows.
  * fc6 emits PSUM [52, (i,s)] directly so the final einsum lhsT needs no
    transpose; its 26 matmuls are split into a 13-a (k-chunk 0) + 13-b
    (k-chunk 1) sequence gated by per-half fc5 sigmoids so the a-half starts
    one sigmoid earlier.

Schedule notes (v2, from the v1 perfetto trace):
  * w6 (286KB bf16) was the body bottleneck in v1: whole-row-contiguous DMAs
    were each pinned to a single DMA engine (~22GB/s), landing at ~16us and
    gating fc6. v2 ships w6 as four 338-column chunks (676B rows, the same
    row size as x, which provably round-robins across all 16 DMA engines)
    spread over the SP and DVE HWDGE rings, all triggered in the preamble.
  * The Activation ring issues NO DMAs: ACT does the sigmoid-table preload
    dummy immediately (gated only on the DVE zb/psum_scr memsets), then is
    free for the sigmoid chain.
  * Pool memsets touch ONLY the ones-rows (disjoint from the sigmoid output
    rows), removing the write-order hazard; PE waits ssem once before conv2.
  * Output: einsum region s is copied PSUM->SBUF by ACT (s=0) / DVE (s=1) in
    parallel, and each region gets its own DRAM store (SP ring / DVE ring).

Raw-bass (Block + explicit semaphores): this walrus build only supports ONE
attached sync-wait per Matmult/DMA instruction, so standalone wait_ge
instructions are used. DMA completion order is not guaranteed across queues,
so consumers wait for the issuing group's FULL credit count (16 per DMA).
"""

import sys
from contextlib import ExitStack

sys.path.insert(0, "/opt/trn_rl_repo")

import numpy as np
import ml_dtypes

# Each DMA delivers 16 completion credits; waiting below 16 proved
# nondeterministic on this runtime, so all consumers wait for the full count.
_DMA_CREDITS = 16

BF16 = ml_dtypes.bfloat16

N_CORES = 8
NS = 2  # sample slots per core
# core i handles samples ASSIGN[i]; host gathers accordingly
ASSIGN = [[0, 8], [1, 9]] + [[i, i] for i in range(2, N_CORES)]

LAST_EXEC_NS = None
LAST_RESULT = None

_BUILT = {}


def _build_nc():
    import concourse.bass as bass
    import concourse.mybir as mybir

    f32 = mybir.dt.float32
    bf16 = mybir.dt.bfloat16
    Sig = mybir.ActivationFunctionType.Sigmoid

    nc = bass.Bass()

    # x rows 0:52 = sample data, row 52 = ones (carries b1 via wb row 52)
    x_d = nc.declare_dram_parameter("x", [64, NS * 168], bf16, isOutput=False)
    # wb: w1b (96 cols, rows 0:53 incl. b1 ones-row) ++ w2b (12, rows 0:25
    # incl. b2 ones-row)
    wb_d = nc.declare_dram_parameter("wb", [64, 108], bf16, isOutput=False)
    # w4p row 6 = b4 in the j=0 block, zeros elsewhere
    w4p_d = nc.declare_dram_parameter("w4p", [12, 720], bf16, isOutput=False)
    w5t_d = nc.declare_dram_parameter("w5t", [86, 200], bf16, isOutput=False)
    # w6 packed: cols 0:676 = k-chunk a (rows 0:100 = w6[:, :100].T),
    # cols 676:1352 = k-chunk b (rows 0:100 = w6[:, 100:200].T, row 100 = b6)
    w6_d = nc.declare_dram_parameter("w6", [106, 1352], bf16, isOutput=False)
    out_d = nc.declare_dram_parameter("out", [13, NS * 168], f32, isOutput=True)

    es = ExitStack()

    def sb(name, shape, dt=bf16):
        return es.enter_context(nc.sbuf_tensor(name, shape, dt))

    def pt(name, shape):
        return es.enter_context(nc.psum_tensor(name, shape, f32))

    with es:
        x_t = sb("x_t", [64, NS * 168])
        wb_t = sb("wb_t", [64, 108])
        w4p_t = sb("w4p_t", [12, 720])
        w5t_t = sb("w5t_t", [86, 200])
        w6_t = sb("w6_t", [106, 1352])
        h1_t = sb("h1_t", [25, NS * 72])   # row 24 = ones (b2 rides w2b row 24)
        h2_t = sb("h2_t", [6, NS * 36])
        tmp6_t = sb("tmp6_t", [6, NS * 18])
        pool_t = sb("pool_t", [7, NS * 9])  # row 6 = ones (b4 rides w4p row 6)
        h4_t = sb("h4_t", [81, NS])         # row 80 = ones (b5 rides w5t row 80)
        t01 = sb("t01", [101, 2 * NS])      # cols 0:2 = t0, 2:4 = t1; row 100 = ones
        h6_t = sb("h6_t", [52, 13 * NS])
        out_t = sb("out_t", [13, NS * 168], f32)
        scr_t = sb("scr_t", [1, 2])         # bf16: table-preload dummy output
        zb_t = sb("zb_t", [101, 1], f32)    # zero bias for all sigmoids

        psum1 = pt("psum1", [24, NS * 72])
        psum2 = pt("psum2", [6, NS * 36])
        psum4 = pt("psum4", [80, NS])
        psum5 = pt("psum5", [100, 2 * NS])
        psum6 = pt("psum6", [52, 13 * NS])
        psume = pt("psume", [13, NS * 168])
        psum_scr = pt("psum_scr", [1, 2])

        dsA = es.enter_context(nc.semaphore("dsA"))    # wb (SP) + x (ACT)
        dsE = es.enter_context(nc.semaphore("dsE"))    # w4p (SWDGE)
        dsF = es.enter_context(nc.semaphore("dsF"))    # w5t (SWDGE)
        dsG = es.enter_context(nc.semaphore("dsG"))    # w6 col-chunks x6 (SP+ACT)
        dsO = es.enter_context(nc.semaphore("dsO"))    # output stores (no waiter)
        ssem = es.enter_context(nc.semaphore("ssem"))  # Pool ones-row memsets
        ssev = es.enter_context(nc.semaphore("ssev"))  # DVE zb/psum_scr memsets
        psem = es.enter_context(nc.semaphore("psem"))
        asem = es.enter_context(nc.semaphore("asem"))
        vsem = es.enter_context(nc.semaphore("vsem"))

        with nc.Block() as block:
            hoist = nc._hoist_insts = []

            @block.sync
            def _(sync):
                # wb first (tiny, conv1's stationary), then five w6 row-range
                # pieces. DMA engine assignment (measured): CONTIGUOUS
                # transfers <64KB round-robin 4-row batches across all 16
                # engines; strided (column-sliced) ones get a 2-engine path;
                # contiguous >=64KB pin to a single engine. Full-width
                # 18-row pieces (48.7KB) hit the fast path.
                hoist.append(sync.dma_start(out=wb_t[:], in_=wb_d[:]).then_inc(dsA, 16))
                for r0 in range(0, 90, 18):
                    hoist.append(
                        sync.dma_start(
                            out=w6_t[r0 : r0 + 18, :], in_=w6_d[r0 : r0 + 18, :]
                        ).then_inc(dsG, 16)
                    )
                # output region 0 once ACT's first copy lands
                sync.wait_ge(asem, 7)
                sync.dma_start(out=out_d[:, 0:168], in_=out_t[:, 0:168]).then_inc(dsO, 16)

            @block.vector
            def _(vector):
                # zb + psum_scr memsets gate only the ACT table preload
                hoist.append(vector.memset(psum_scr[:], 0.0).then_inc(ssev))
                hoist.append(vector.memset(zb_t[:], 0.0).then_inc(ssev))
                # (h, w) pooling as ONE 4-tap reduce over the (dh, dw) dims
                vector.wait_ge(ssem, 4)
                vector.wait_ge(asem, 2)
                h2r = h2_t[:].rearrange(
                    "p (s hp dh wp dw) -> p (s hp) wp dh dw", s=NS, hp=3, dh=2, wp=3, dw=2
                )
                poolr = pool_t[0:6, :].rearrange("p (s hp wp) -> p (s hp) wp", s=NS, hp=3, wp=3)
                with nc.allow_low_precision("4-term bf16 pooling sum, matches prior impl"):
                    vector.tensor_reduce(
                        poolr[:], h2r[:], axis=mybir.AxisListType.XY, op=mybir.AluOpType.add
                    ).then_inc(vsem)  # 1


            @block.gpsimd
            def _(gpsimd):
                # weight DMAs first (SWDGE gen is ~870ns each), then the
                # ones-row memsets (disjoint from all sigmoid output rows)
                hoist.append(gpsimd.dma_start(out=w4p_t[:], in_=w4p_d[:]).then_inc(dsE, 16))
                hoist.append(gpsimd.dma_start(out=w5t_t[:], in_=w5t_d[:]).then_inc(dsF, 16))
                # whole-tensor ones memsets (APs must start at partition 0);
                # writers of the non-ones rows wait on ssem first
                hoist.append(gpsimd.memset(h1_t[:], 1.0).then_inc(ssem))
                hoist.append(gpsimd.memset(pool_t[:], 1.0).then_inc(ssem))
                hoist.append(gpsimd.memset(h4_t[:], 1.0).then_inc(ssem))
                hoist.append(gpsimd.memset(t01[:], 1.0).then_inc(ssem))

            @block.scalar
            def _(scalar):
                # x rides the ACT ring first-thing: the ACT engine starts
                # ~260ns before SP, so conv1's gate lands earliest here.
                # The last w6 piece follows while the table loads.
                hoist.append(scalar.dma_start(out=x_t[:], in_=x_d[:]).then_inc(dsA, 16))
                hoist.append(
                    scalar.dma_start(out=w6_t[90:106, :], in_=w6_d[90:106, :]).then_inc(dsG, 16)
                )
                scalar.wait_ge(ssev, 2)
                # dummy sigmoid FIRST IN THIS BASIC BLOCK: walrus tracks ACT
                # tables per-bb, so the preload must live in the same bb as
                # the real sigmoids to avoid a 1.3us reload before sig1
                scalar.activation(scr_t[:], psum_scr[:], Sig, bias=zb_t[0:1, :])
                scalar.wait_ge(ssem, 4)
                scalar.wait_ge(psem, 1)
                scalar.activation(h1_t[0:24, :], psum1[:], Sig, bias=zb_t[0:24, :]).then_inc(asem)  # 1
                scalar.wait_ge(psem, 2)
                scalar.activation(h2_t[:], psum2[:], Sig, bias=zb_t[0:6, :]).then_inc(asem)  # 2
                scalar.wait_ge(psem, 3)
                scalar.activation(h4_t[0:80, :], psum4[:], Sig, bias=zb_t[0:80, :]).then_inc(asem)  # 3
                scalar.wait_ge(psem, 4)
                scalar.activation(
                    t01[0:100, 0:NS], psum5[:, 0:NS], Sig, bias=zb_t[0:100, :]
                ).then_inc(asem)  # 4
                scalar.wait_ge(psem, 5)
                scalar.activation(
                    t01[0:100, NS : 2 * NS], psum5[:, NS : 2 * NS], Sig, bias=zb_t[0:100, :]
                ).then_inc(asem)  # 5
                scalar.wait_ge(psem, 6)
                scalar.activation(h6_t[:], psum6[:], Sig, bias=zb_t[0:52, :]).then_inc(asem)  # 6
                scalar.wait_ge(psem, 7)
                scalar.copy(out_t[:, 0:168], psume[:, 0:168]).then_inc(asem)  # 7
                scalar.wait_ge(psem, 8)
                scalar.copy(out_t[:, 168:336], psume[:, 168:336]).then_inc(asem)  # 8
                # region-1 store on ACT's own ring; same-engine ordering still
                # needs the semaphore round-trip (DMA data fetch is async)
                scalar.wait_ge(asem, 8)
                scalar.dma_start(out=out_d[:, 168:336], in_=out_t[:, 168:336]).then_inc(dsO, 16)

            @block.tensor
            def _(tensor):
                # conv1: 4 accumulated matmuls; K=53 incl. the b1 ones-row
                tensor.wait_ge(dsA, 2 * _DMA_CREDITS)  # wb (SP) + x (ACT)
                x4 = x_t[0:53, :].rearrange("p (s h w) -> p s h w", s=NS, h=7, w=24)
                taps1 = [(kh, kw) for kh in range(2) for kw in range(2)]
                for k, (kh, kw) in enumerate(taps1):
                    mm = tensor.matmul(
                        psum1[:],
                        wb_t[0:53, k * 24 : (k + 1) * 24],
                        x4[:, :, kh : kh + 6, kw : kw + 23 : 2],
                        start=(k == 0),
                        stop=(k == 3),
                    )
                    if k == 3:
                        mm.then_inc(psem)  # psem 1
                # conv2: K=25 incl. the b2 ones-row. asem>=1 transitively
                # covers the Pool ones memsets (ACT waits ssem before sig1).
                tensor.wait_ge(asem, 1)
                h14 = h1_t[:].rearrange("p (s h w) -> p s h w", s=NS, h=6, w=12)
                for kw in range(2):
                    mm = tensor.matmul(
                        psum2[:],
                        wb_t[0:25, 96 + kw * 6 : 96 + (kw + 1) * 6],
                        h14[:, :, :, kw : kw + 11 : 2],
                        start=(kw == 0),
                        stop=(kw == 1),
                    )
                    if kw == 1:
                        mm.then_inc(psem)  # psem 2
                # fc4: 9 (hp,wp) matmuls vs the h/w-pooled tile; d-pooling and
                # /8 live in w4p; j=0 has K=7 incl. the b4 ones-row
                tensor.wait_ge(vsem, 1)
                tensor.wait_ge(dsE, 16)
                pool4 = pool_t[:].rearrange("p (s j) -> p s j", s=NS, j=9)
                for j in range(9):
                    kk = 7 if j == 0 else 6
                    mm = tensor.matmul(
                        psum4[:],
                        w4p_t[0:kk, j * 80 : (j + 1) * 80],
                        pool4[0:kk, :, j],
                        start=(j == 0),
                        stop=(j == 8),
                    )
                    if j == 8:
                        mm.then_inc(psem)  # psem 3
                # fc5: two k-halves, each gating its own sigmoid so fc6's
                # a-matmuls can start one ACT op earlier
                tensor.wait_ge(asem, 3)
                tensor.wait_ge(dsF, 16)
                tensor.matmul(
                    psum5[:, 0:NS], w5t_t[0:81, 0:100], h4_t[:], start=True, stop=True
                ).then_inc(psem)  # psem 4
                tensor.matmul(
                    psum5[:, NS : 2 * NS], w5t_t[0:81, 100:200], h4_t[:], start=True, stop=True
                ).then_inc(psem)  # psem 5
                # fc6: 13 i-chunks x 2 k-chunks (a_i, b_i interleaved — only
                # one PSUM accumulation group may be open per zero region)
                tensor.wait_ge(asem, 5)
                tensor.wait_ge(dsG, 96)
                for i in range(13):
                    tensor.matmul(
                        psum6[:, i * NS : (i + 1) * NS],
                        w6_t[0:100, i * 52 : (i + 1) * 52],
                        t01[0:100, 0:NS],
                        start=True,
                        stop=False,
                    )
                    mm = tensor.matmul(
                        psum6[:, i * NS : (i + 1) * NS],
                        w6_t[0:101, 676 + i * 52 : 676 + (i + 1) * 52],
                        t01[:, NS : 2 * NS],
                        start=False,
                        stop=True,
                    )
                    if i == 12:
                        mm.then_inc(psem)  # psem 6
                # einsum; each sample region releases its own copy engine
                tensor.wait_ge(asem, 6)
                h6v = h6_t[:].rearrange("p (i s) -> p s i", s=NS)
                for s in range(NS):
                    tensor.matmul(
                        psume[:, s * 168 : (s + 1) * 168],
                        h6v[:, s, :],
                        x_t[0:52, s * 168 : (s + 1) * 168],
                        start=True,
                        stop=True,
                    ).then_inc(psem)  # psem 7, 8

    _strip_entry_barrier(nc)
    return nc


def _strip_entry_barrier(nc):
    f = nc.m.functions[0]
    bbs = {bb.name: bb for bb in f.blocks}
    main = bbs["main"]
    # 1) drop the init all-engine barrier (nothing reads the const-AP tiles)
    main.instructions = [
        i
        for i in main.instructions
        if not (
            i.name.startswith("barrier_")
            or getattr(i, "opcode", "") == "Drain"
            or type(i).__name__ == "InstDrain"
        )
    ]
    # 2) hoist the input-DMA triggers into main so transfers start during the
    #    preamble, before the Block-entry rendezvous
    hoisted = {bi.ins.name for bi in getattr(nc, "_hoist_insts", [])}
    if hoisted:
        moved = []
        for bb in f.blocks:
            if bb.name == "main" or not bb.instructions:
                continue
            keep = []
            for i in bb.instructions:
                (moved if i.name in hoisted else keep).append(i)
            if len(keep) != len(bb.instructions):
                bb.instructions = keep
        # insert at the very top of main (after the entry Call): the DMA
        # triggers use only immediates + the parameter table, not the
        # preamble registers
        insts = main.instructions
        main.instructions = insts[:1] + moved + insts[1:]


def _prep_weights(w1, b1, w2, b2, w4, b4, w5, b5, w6, b6):
    f = np.float32
    w1v = np.asarray(w1, f)[0, 0]  # (6,2,2)
    w2v = np.asarray(w2, f)[0, 0, :, 0, :]  # (4,2)
    w4 = np.asarray(w4, f)
    w5 = np.asarray(w5, f)
    w6 = np.asarray(w6, f)
    b1 = np.asarray(b1, f)
    b2 = np.asarray(b2, f)
    b4 = np.asarray(b4, f)
    b5 = np.asarray(b5, f)
    b6 = np.asarray(b6, f)

    wb = np.zeros((64, 108), f)
    for kd in range(6):
        for kh in range(2):
            for kw in range(2):
                for d in range(24):
                    wb[2 * d + kd, (kh * 2 + kw) * 24 + d] = w1v[kd, kh, kw]
    wb[52, 0:24] = b1[0]  # ones-row bias, k=0 tap block only
    for kd in range(4):
        for kw in range(2):
            for d in range(6):
                wb[4 * d + kd, 96 + kw * 6 + d] = w2v[kd, kw]
    wb[24, 96:102] = b2[0]  # ones-row bias, kw=0 block only

    w4r = w4.reshape(80, 3, 3, 3) / 8.0
    w4q = np.transpose(w4r, (1, 2, 3, 0)).reshape(3, 720)
    w4p = np.zeros((12, 720), f)
    w4p[0:6:2, :] = w4q
    w4p[1:6:2, :] = w4q
    w4p[6, 0:80] = b4  # ones-row bias, j=0 block only

    w5t = np.zeros((86, 200), f)
    w5t[0:80, :] = w5.T
    w5t[80, :] = b5

    w6p = np.zeros((106, 1352), f)
    w6p[0:100, 0:676] = w6[:, 0:100].T
    w6p[0:100, 676:1352] = w6[:, 100:200].T
    w6p[100, 676:1352] = b6

    return dict(
        wb=wb.astype(BF16),
        w4p=w4p.astype(BF16),
        w5t=w5t.astype(BF16),
        w6=w6p.astype(BF16),
    )


def kernel(x, w1, b1, w2, b2, w4, b4, w5, b5, w6, b6, _trace=False):
    global LAST_EXEC_NS, LAST_RESULT
    from concourse.bass_utils import run_bass_kernel_spmd

    if "nc" not in _BUILT:
        _BUILT["nc"] = _build_nc()
    nc = _BUILT["nc"]

    xs = np.ascontiguousarray(np.asarray(x, np.float32).reshape(10, 52, 168))
    wd = _prep_weights(w1, b1, w2, b2, w4, b4, w5, b5, w6, b6)

    in_maps = []
    for i in range(N_CORES):
        xc = np.ones((64, NS * 168), np.float32)
        xc[0:52] = np.transpose(np.stack([xs[a] for a in ASSIGN[i]]), (1, 0, 2)).reshape(52, NS * 168)
        xc = np.ascontiguousarray(xc.astype(BF16))
        m = {"x": xc}
        m.update(wd)
        in_maps.append(m)

    res = run_bass_kernel_spmd(nc, in_maps, core_ids=list(range(N_CORES)), trace=_trace)
    LAST_EXEC_NS = res.exec_time_ns
    LAST_RESULT = res

    out = np.zeros((10, 2184), np.float32)
    for i in range(N_CORES):
        o = res.results[i]["out"].reshape(13, NS, 168)
        for slot, b in enumerate(ASSIGN[i]):
            out[b] = o[:, slot, :].reshape(2184)
    return out


# revision 30
# speedup vs baseline: 1.1854x; 1.0478x over previous
"""Trainium2 Bass kernel for nn_C3DNet — data-parallel over the 10 samples on 8 cores.

Math (per sample, from the reference):
  x:(52,7,24) -conv1(6,2,2)s(2,1,2)+sig-> (24,6,12) -conv2(4,1,2)s(4,1,2)+sig-> (6,6,6)
  -avgpool2-> 27 -fc4+sig-> 80 -fc5+sig-> 200 -fc6+sig-> 676
  out = h6.reshape(13,52) @ x.reshape(52,168)  -> (13,168) -> 2184

Everything is cast as TensorE matmuls (bf16 datapath, f32 PSUM):
  * conv1/conv2 contract the D dimension (on partitions) using host-built
    banded weight matrices; the (h,w) taps become strided free-dim views.
  * fc4 contracts q=3 partitions x 9 (hp,wp) matmuls; b1/b2/b4 applied via the
    ACT sigmoid's per-partition bias operand; b5/b6 folded via ones-rows.
  * fc6 emits PSUM [52, (i,s)] directly so the final einsum lhsT needs no
    transpose; its 26 matmuls are split into a 13-a (k-chunk 0) + 13-b
    (k-chunk 1) sequence gated by per-half fc5 sigmoids so the a-half starts
    one sigmoid earlier.

Schedule notes (v2, from the v1 perfetto trace):
  * w6 (286KB bf16) was the body bottleneck in v1: whole-row-contiguous DMAs
    were each pinned to a single DMA engine (~22GB/s), landing at ~16us and
    gating fc6. v2 ships w6 as four 338-column chunks (676B rows, the same
    row size as x, which provably round-robins across all 16 DMA engines)
    spread over the SP and DVE HWDGE rings, all triggered in the preamble.
  * The Activation ring issues NO DMAs: ACT does the sigmoid-table preload
    dummy immediately (gated only on the DVE zb/psum_scr memsets), then is
    free for the sigmoid chain.
  * Pool memsets touch ONLY the ones-rows (disjoint from the sigmoid output
    rows), removing the write-order hazard; PE waits ssem once before conv2.
  * Output: einsum region s is copied PSUM->SBUF by ACT (s=0) / DVE (s=1) in
    parallel, and each region gets its own DRAM store (SP ring / DVE ring).

Raw-bass (Block + explicit semaphores): this walrus build only supports ONE
attached sync-wait per Matmult/DMA instruction, so standalone wait_ge
instructions are used. DMA completion order is not guaranteed across queues,
so consumers wait for the issuing group's FULL credit count (16 per DMA).
"""

import sys
from contextlib import ExitStack

sys.path.insert(0, "/opt/trn_rl_repo")

import numpy as np
import ml_dtypes

# Each DMA delivers 16 completion credits; waiting below 16 proved
# nondeterministic on this runtime, so all consumers wait for the full count.
_DMA_CREDITS = 16

BF16 = ml_dtypes.bfloat16

N_CORES = 8
NS = 2  # sample slots per core
# core i handles samples ASSIGN[i]; host gathers accordingly
ASSIGN = [[0, 8], [1, 9]] + [[i, i] for i in range(2, N_CORES)]

LAST_EXEC_NS = None
LAST_RESULT = None

_BUILT = {}


def _build_nc():
    import concourse.bass as bass
    import concourse.mybir as mybir

    f32 = mybir.dt.float32
    bf16 = mybir.dt.bfloat16
    Sig = mybir.ActivationFunctionType.Sigmoid

    nc = bass.Bass()

    # x rows 0:52 = sample data, row 52 = ones (carries b1 via wb row 52)
    x_d = nc.declare_dram_parameter("x", [64, NS * 168], bf16, isOutput=False)
    # wb: w1b (96 cols, rows 0:53 incl. b1 ones-row) ++ w2b (12, rows 0:25
    # incl. b2 ones-row)
    wb_d = nc.declare_dram_parameter("wb", [64, 108], bf16, isOutput=False)
    # w4p row 6 = b4 in the j=0 block, zeros elsewhere
    w4p_d = nc.declare_dram_parameter("w4p", [12, 720], bf16, isOutput=False)
    w5t_d = nc.declare_dram_parameter("w5t", [86, 200], bf16, isOutput=False)
    # w6 packed: cols 0:676 = k-chunk a (rows 0:100 = w6[:, :100].T),
    # cols 676:1352 = k-chunk b (rows 0:100 = w6[:, 100:200].T, row 100 = b6)
    w6_d = nc.declare_dram_parameter("w6", [106, 1352], bf16, isOutput=False)
    out_d = nc.declare_dram_parameter("out", [13, NS * 168], f32, isOutput=True)

    es = ExitStack()

    def sb(name, shape, dt=bf16):
        return es.enter_context(nc.sbuf_tensor(name, shape, dt))

    def pt(name, shape):
        return es.enter_context(nc.psum_tensor(name, shape, f32))

    with es:
        x_t = sb("x_t", [64, NS * 168])
        wb_t = sb("wb_t", [64, 108])
        w4p_t = sb("w4p_t", [12, 720])
        w5t_t = sb("w5t_t", [86, 200])
        w6_t = sb("w6_t", [106, 1352])
        h1_t = sb("h1_t", [25, NS * 72])   # row 24 = ones (b2 rides w2b row 24)
        h2_t = sb("h2_t", [6, NS * 36])
        tmp6_t = sb("tmp6_t", [6, NS * 18])
        pool_t = sb("pool_t", [7, NS * 9])  # row 6 = ones (b4 rides w4p row 6)
        h4_t = sb("h4_t", [81, NS])         # row 80 = ones (b5 rides w5t row 80)
        t01 = sb("t01", [101, 2 * NS])      # cols 0:2 = t0, 2:4 = t1; row 100 = ones
        h6_t = sb("h6_t", [52, 13 * NS])
        out_t = sb("out_t", [13, NS * 168], f32)
        scr_t = sb("scr_t", [1, 2])         # bf16: table-preload dummy output
        zb_t = sb("zb_t", [101, 1], f32)    # zero bias for all sigmoids

        psum1 = pt("psum1", [24, NS * 72])
        psum2 = pt("psum2", [6, NS * 36])
        psum4 = pt("psum4", [80, NS])
        psum5 = pt("psum5", [100, 2 * NS])
        psum6 = pt("psum6", [52, 13 * NS])
        psume = pt("psume", [13, NS * 168])
        psum_scr = pt("psum_scr", [1, 2])

        dsA = es.enter_context(nc.semaphore("dsA"))    # wb (SP) + x (ACT)
        dsE = es.enter_context(nc.semaphore("dsE"))    # w4p (SWDGE)
        dsF = es.enter_context(nc.semaphore("dsF"))    # w5t (SWDGE)
        dsG = es.enter_context(nc.semaphore("dsG"))    # w6 col-chunks x6 (SP+ACT)
        dsO = es.enter_context(nc.semaphore("dsO"))    # output stores (no waiter)
        ssem = es.enter_context(nc.semaphore("ssem"))  # Pool ones-row memsets
        ssev = es.enter_context(nc.semaphore("ssev"))  # DVE zb/psum_scr memsets
        psem = es.enter_context(nc.semaphore("psem"))
        asem = es.enter_context(nc.semaphore("asem"))
        vsem = es.enter_context(nc.semaphore("vsem"))

        with nc.Block() as block:
            hoist = nc._hoist_insts = []

            @block.sync
            def _(sync):
                # wb first (tiny, conv1's stationary), then five w6 row-range
                # pieces. DMA engine assignment (measured): CONTIGUOUS
                # transfers <64KB round-robin 4-row batches across all 16
                # engines; strided (column-sliced) ones get a 2-engine path;
                # contiguous >=64KB pin to a single engine. Full-width
                # 18-row pieces (48.7KB) hit the fast path.
                hoist.append(sync.dma_start(out=wb_t[:], in_=wb_d[:]).then_inc(dsA, 16))
                hoist.append(sync.dma_start(out=w5t_t[:], in_=w5t_d[:]).then_inc(dsF, 16))
                for r0 in range(0, 90, 18):
                    hoist.append(
                        sync.dma_start(
                            out=w6_t[r0 : r0 + 18, :], in_=w6_d[r0 : r0 + 18, :]
                        ).then_inc(dsG, 16)
                    )
                # single full-width output store once both ACT copies land
                sync.wait_ge(asem, 8)
                sync.dma_start(out=out_d[:, :], in_=out_t[:]).then_inc(dsO, 16)

            @block.vector
            def _(vector):
                # zb + psum_scr memsets gate only the ACT table preload
                hoist.append(vector.memset(psum_scr[:], 0.0).then_inc(ssev))
                hoist.append(vector.memset(zb_t[:], 0.0).then_inc(ssev))
                # (h, w) pooling as ONE 4-tap reduce over the (dh, dw) dims
                vector.wait_ge(ssem, 4)
                vector.wait_ge(asem, 2)
                h2r = h2_t[:].rearrange(
                    "p (s hp dh wp dw) -> p (s hp) wp dh dw", s=NS, hp=3, dh=2, wp=3, dw=2
                )
                poolr = pool_t[0:6, :].rearrange("p (s hp wp) -> p (s hp) wp", s=NS, hp=3, wp=3)
                with nc.allow_low_precision("4-term bf16 pooling sum, matches prior impl"):
                    vector.tensor_reduce(
                        poolr[:], h2r[:], axis=mybir.AxisListType.XY, op=mybir.AluOpType.add
                    ).then_inc(vsem)  # 1


            @block.gpsimd
            def _(gpsimd):
                # ones memsets first (sig1 waits ssem at ~3.4us), then w4p
                # (needed ~5.5us) and the last w6 piece (needed ~7.4us).
                # Whole-tensor memsets: APs must start at partition 0;
                # writers of the non-ones rows wait on ssem first.
                hoist.append(gpsimd.memset(h1_t[:], 1.0).then_inc(ssem))
                hoist.append(gpsimd.memset(pool_t[:], 1.0).then_inc(ssem))
                hoist.append(gpsimd.memset(h4_t[:], 1.0).then_inc(ssem))
                hoist.append(gpsimd.memset(t01[:], 1.0).then_inc(ssem))
                hoist.append(gpsimd.dma_start(out=w4p_t[:], in_=w4p_d[:]).then_inc(dsE, 16))
                hoist.append(
                    gpsimd.dma_start(out=w6_t[90:106, :], in_=w6_d[90:106, :]).then_inc(dsG, 16)
                )

            @block.scalar
            def _(scalar):
                # x rides the ACT ring alone: the ACT engine starts ~260ns
                # before SP, so conv1's gate lands earliest here, and the
                # table-load dummy follows with no trigger in its way.
                hoist.append(scalar.dma_start(out=x_t[:], in_=x_d[:]).then_inc(dsA, 16))
                scalar.wait_ge(ssev, 2)
                # dummy sigmoid FIRST IN THIS BASIC BLOCK: walrus tracks ACT
                # tables per-bb, so the preload must live in the same bb as
                # the real sigmoids to avoid a 1.3us reload before sig1
                scalar.activation(scr_t[:], psum_scr[:], Sig, bias=zb_t[0:1, :])
                scalar.wait_ge(ssem, 4)
                scalar.wait_ge(psem, 1)
                scalar.activation(h1_t[0:24, :], psum1[:], Sig, bias=zb_t[0:24, :]).then_inc(asem)  # 1
                scalar.wait_ge(psem, 2)
                scalar.activation(h2_t[:], psum2[:], Sig, bias=zb_t[0:6, :]).then_inc(asem)  # 2
                scalar.wait_ge(psem, 3)
                scalar.activation(h4_t[0:80, :], psum4[:], Sig, bias=zb_t[0:80, :]).then_inc(asem)  # 3
                scalar.wait_ge(psem, 4)
                scalar.activation(
                    t01[0:100, 0:NS], psum5[:, 0:NS], Sig, bias=zb_t[0:100, :]
                ).then_inc(asem)  # 4
                scalar.wait_ge(psem, 5)
                scalar.activation(
                    t01[0:100, NS : 2 * NS], psum5[:, NS : 2 * NS], Sig, bias=zb_t[0:100, :]
                ).then_inc(asem)  # 5
                scalar.wait_ge(psem, 6)
                scalar.activation(h6_t[:], psum6[:], Sig, bias=zb_t[0:52, :]).then_inc(asem)  # 6
                scalar.wait_ge(psem, 7)
                scalar.copy(out_t[:, 0:168], psume[:, 0:168]).then_inc(asem)  # 7
                scalar.wait_ge(psem, 8)
                scalar.copy(out_t[:, 168:336], psume[:, 168:336]).then_inc(asem)  # 8

            @block.tensor
            def _(tensor):
                # conv1: 4 accumulated matmuls; K=53 incl. the b1 ones-row
                tensor.wait_ge(dsA, 2 * _DMA_CREDITS)  # wb (SP) + x (ACT)
                x4 = x_t[0:53, :].rearrange("p (s h w) -> p s h w", s=NS, h=7, w=24)
                taps1 = [(kh, kw) for kh in range(2) for kw in range(2)]
                for k, (kh, kw) in enumerate(taps1):
                    mm = tensor.matmul(
                        psum1[:],
                        wb_t[0:53, k * 24 : (k + 1) * 24],
                        x4[:, :, kh : kh + 6, kw : kw + 23 : 2],
                        start=(k == 0),
                        stop=(k == 3),
                    )
                    if k == 3:
                        mm.then_inc(psem)  # psem 1
                # conv2: K=25 incl. the b2 ones-row. asem>=1 transitively
                # covers the Pool ones memsets (ACT waits ssem before sig1).
                tensor.wait_ge(asem, 1)
                h14 = h1_t[:].rearrange("p (s h w) -> p s h w", s=NS, h=6, w=12)
                for kw in range(2):
                    mm = tensor.matmul(
                        psum2[:],
                        wb_t[0:25, 96 + kw * 6 : 96 + (kw + 1) * 6],
                        h14[:, :, :, kw : kw + 11 : 2],
                        start=(kw == 0),
                        stop=(kw == 1),
                    )
                    if kw == 1:
                        mm.then_inc(psem)  # psem 2
                # fc4: 9 (hp,wp) matmuls vs the h/w-pooled tile; d-pooling and
                # /8 live in w4p; j=0 has K=7 incl. the b4 ones-row
                tensor.wait_ge(vsem, 1)
                tensor.wait_ge(dsE, 16)
                pool4 = pool_t[:].rearrange("p (s j) -> p s j", s=NS, j=9)
                for j in range(9):
                    kk = 7 if j == 0 else 6
                    mm = tensor.matmul(
                        psum4[:],
                        w4p_t[0:kk, j * 80 : (j + 1) * 80],
                        pool4[0:kk, :, j],
                        start=(j == 0),
                        stop=(j == 8),
                    )
                    if j == 8:
                        mm.then_inc(psem)  # psem 3
                # fc5: two k-halves, each gating its own sigmoid so fc6's
                # a-matmuls can start one ACT op earlier
                tensor.wait_ge(asem, 3)
                tensor.wait_ge(dsF, 16)
                tensor.matmul(
                    psum5[:, 0:NS], w5t_t[0:81, 0:100], h4_t[:], start=True, stop=True
                ).then_inc(psem)  # psem 4
                tensor.matmul(
                    psum5[:, NS : 2 * NS], w5t_t[0:81, 100:200], h4_t[:], start=True, stop=True
                ).then_inc(psem)  # psem 5
                # fc6: 13 i-chunks x 2 k-chunks (a_i, b_i interleaved — only
                # one PSUM accumulation group may be open per zero region)
                tensor.wait_ge(asem, 5)
                tensor.wait_ge(dsG, 96)
                for i in range(13):
                    tensor.matmul(
                        psum6[:, i * NS : (i + 1) * NS],
                        w6_t[0:100, i * 52 : (i + 1) * 52],
                        t01[0:100, 0:NS],
                        start=True,
                        stop=False,
                    )
                    mm = tensor.matmul(
                        psum6[:, i * NS : (i + 1) * NS],
                        w6_t[0:101, 676 + i * 52 : 676 + (i + 1) * 52],
                        t01[:, NS : 2 * NS],
                        start=False,
                        stop=True,
                    )
                    if i == 12:
                        mm.then_inc(psem)  # psem 6
                # einsum; each sample region releases its own copy engine
                tensor.wait_ge(asem, 6)
                h6v = h6_t[:].rearrange("p (i s) -> p s i", s=NS)
                for s in range(NS):
                    tensor.matmul(
                        psume[:, s * 168 : (s + 1) * 168],
                        h6v[:, s, :],
                        x_t[0:52, s * 168 : (s + 1) * 168],
                        start=True,
                        stop=True,
                    ).then_inc(psem)  # psem 7, 8

    _strip_entry_barrier(nc)
    return nc


def _strip_entry_barrier(nc):
    f = nc.m.functions[0]
    bbs = {bb.name: bb for bb in f.blocks}
    main = bbs["main"]
    # 1) drop the init all-engine barrier (nothing reads the const-AP tiles)
    main.instructions = [
        i
        for i in main.instructions
        if not (
            i.name.startswith("barrier_")
            or getattr(i, "opcode", "") == "Drain"
            or type(i).__name__ == "InstDrain"
        )
    ]
    # 2) hoist the input-DMA triggers into main so transfers start during the
    #    preamble, before the Block-entry rendezvous
    hoisted = {bi.ins.name for bi in getattr(nc, "_hoist_insts", [])}
    if hoisted:
        moved = []
        for bb in f.blocks:
            if bb.name == "main" or not bb.instructions:
                continue
            keep = []
            for i in bb.instructions:
                (moved if i.name in hoisted else keep).append(i)
            if len(keep) != len(bb.instructions):
                bb.instructions = keep
        # insert at the very top of main (after the entry Call): the DMA
        # triggers use only immediates + the parameter table, not the
        # preamble registers
        insts = main.instructions
        main.instructions = insts[:1] + moved + insts[1:]


def _prep_weights(w1, b1, w2, b2, w4, b4, w5, b5, w6, b6):
    f = np.float32
    w1v = np.asarray(w1, f)[0, 0]  # (6,2,2)
    w2v = np.asarray(w2, f)[0, 0, :, 0, :]  # (4,2)
    w4 = np.asarray(w4, f)
    w5 = np.asarray(w5, f)
    w6 = np.asarray(w6, f)
    b1 = np.asarray(b1, f)
    b2 = np.asarray(b2, f)
    b4 = np.asarray(b4, f)
    b5 = np.asarray(b5, f)
    b6 = np.asarray(b6, f)

    wb = np.zeros((64, 108), f)
    for kd in range(6):
        for kh in range(2):
            for kw in range(2):
                for d in range(24):
                    wb[2 * d + kd, (kh * 2 + kw) * 24 + d] = w1v[kd, kh, kw]
    wb[52, 0:24] = b1[0]  # ones-row bias, k=0 tap block only
    for kd in range(4):
        for kw in range(2):
            for d in range(6):
                wb[4 * d + kd, 96 + kw * 6 + d] = w2v[kd, kw]
    wb[24, 96:102] = b2[0]  # ones-row bias, kw=0 block only

    w4r = w4.reshape(80, 3, 3, 3) / 8.0
    w4q = np.transpose(w4r, (1, 2, 3, 0)).reshape(3, 720)
    w4p = np.zeros((12, 720), f)
    w4p[0:6:2, :] = w4q
    w4p[1:6:2, :] = w4q
    w4p[6, 0:80] = b4  # ones-row bias, j=0 block only

    w5t = np.zeros((86, 200), f)
    w5t[0:80, :] = w5.T
    w5t[80, :] = b5

    w6p = np.zeros((106, 1352), f)
    w6p[0:100, 0:676] = w6[:, 0:100].T
    w6p[0:100, 676:1352] = w6[:, 100:200].T
    w6p[100, 676:1352] = b6

    return dict(
        wb=wb.astype(BF16),
        w4p=w4p.astype(BF16),
        w5t=w5t.astype(BF16),
        w6=w6p.astype(BF16),
    )


def kernel(x, w1, b1, w2, b2, w4, b4, w5, b5, w6, b6, _trace=False):
    global LAST_EXEC_NS, LAST_RESULT
    from concourse.bass_utils import run_bass_kernel_spmd

    if "nc" not in _BUILT:
        _BUILT["nc"] = _build_nc()
    nc = _BUILT["nc"]

    xs = np.ascontiguousarray(np.asarray(x, np.float32).reshape(10, 52, 168))
    wd = _prep_weights(w1, b1, w2, b2, w4, b4, w5, b5, w6, b6)

    in_maps = []
    for i in range(N_CORES):
        xc = np.ones((64, NS * 168), np.float32)
        xc[0:52] = np.transpose(np.stack([xs[a] for a in ASSIGN[i]]), (1, 0, 2)).reshape(52, NS * 168)
        xc = np.ascontiguousarray(xc.astype(BF16))
        m = {"x": xc}
        m.update(wd)
        in_maps.append(m)

    res = run_bass_kernel_spmd(nc, in_maps, core_ids=list(range(N_CORES)), trace=_trace)
    LAST_EXEC_NS = res.exec_time_ns
    LAST_RESULT = res

    out = np.zeros((10, 2184), np.float32)
    for i in range(N_CORES):
        o = res.results[i]["out"].reshape(13, NS, 168)
        for slot, b in enumerate(ASSIGN[i]):
            out[b] = o[:, slot, :].reshape(2184)
    return out


# revision 33
# speedup vs baseline: 1.1962x; 1.0091x over previous
"""Trainium2 Bass kernel for nn_C3DNet — data-parallel over the 10 samples on 8 cores.

Math (per sample, from the reference):
  x:(52,7,24) -conv1(6,2,2)s(2,1,2)+sig-> (24,6,12) -conv2(4,1,2)s(4,1,2)+sig-> (6,6,6)
  -avgpool2-> 27 -fc4+sig-> 80 -fc5+sig-> 200 -fc6+sig-> 676
  out = h6.reshape(13,52) @ x.reshape(52,168)  -> (13,168) -> 2184

Everything is cast as TensorE matmuls (bf16 datapath, f32 PSUM):
  * conv1/conv2 contract the D dimension (on partitions) using host-built
    banded weight matrices; the (h,w) taps become strided free-dim views.
  * fc4 contracts q=3 partitions x 9 (hp,wp) matmuls; b1/b2/b4 applied via the
    ACT sigmoid's per-partition bias operand; b5/b6 folded via ones-rows.
  * fc6 emits PSUM [52, (i,s)] directly so the final einsum lhsT needs no
    transpose; its 26 matmuls are split into a 13-a (k-chunk 0) + 13-b
    (k-chunk 1) sequence gated by per-half fc5 sigmoids so the a-half starts
    one sigmoid earlier.

Schedule notes (v2, from the v1 perfetto trace):
  * w6 (286KB bf16) was the body bottleneck in v1: whole-row-contiguous DMAs
    were each pinned to a single DMA engine (~22GB/s), landing at ~16us and
    gating fc6. v2 ships w6 as four 338-column chunks (676B rows, the same
    row size as x, which provably round-robins across all 16 DMA engines)
    spread over the SP and DVE HWDGE rings, all triggered in the preamble.
  * The Activation ring issues NO DMAs: ACT does the sigmoid-table preload
    dummy immediately (gated only on the DVE zb/psum_scr memsets), then is
    free for the sigmoid chain.
  * Pool memsets touch ONLY the ones-rows (disjoint from the sigmoid output
    rows), removing the write-order hazard; PE waits ssem once before conv2.
  * Output: einsum region s is copied PSUM->SBUF by ACT (s=0) / DVE (s=1) in
    parallel, and each region gets its own DRAM store (SP ring / DVE ring).

Raw-bass (Block + explicit semaphores): this walrus build only supports ONE
attached sync-wait per Matmult/DMA instruction, so standalone wait_ge
instructions are used. DMA completion order is not guaranteed across queues,
so consumers wait for the issuing group's FULL credit count (16 per DMA).
"""

import sys
from contextlib import ExitStack

sys.path.insert(0, "/opt/trn_rl_repo")

import numpy as np
import ml_dtypes

# Each DMA delivers 16 completion credits; waiting below 16 proved
# nondeterministic on this runtime, so all consumers wait for the full count.
_DMA_CREDITS = 16

BF16 = ml_dtypes.bfloat16

N_CORES = 8
NS = 2  # sample slots per core
# core i handles samples ASSIGN[i]; host gathers accordingly
ASSIGN = [[0, 8], [1, 9]] + [[i, i] for i in range(2, N_CORES)]

LAST_EXEC_NS = None
LAST_RESULT = None

_BUILT = {}


def _build_nc():
    import concourse.bass as bass
    import concourse.mybir as mybir

    f32 = mybir.dt.float32
    bf16 = mybir.dt.bfloat16
    Sig = mybir.ActivationFunctionType.Sigmoid

    nc = bass.Bass()

    # x rows 0:52 = sample data, row 52 = ones (carries b1 via wb row 52)
    x_d = nc.declare_dram_parameter("x", [64, NS * 168], bf16, isOutput=False)
    # wb: w1b (96 cols, rows 0:53 incl. b1 ones-row) ++ w2b (12, rows 0:25
    # incl. b2 ones-row)
    wb_d = nc.declare_dram_parameter("wb", [64, 108], bf16, isOutput=False)
    # w4p row 6 = b4 in the j=0 block, zeros elsewhere
    w4p_d = nc.declare_dram_parameter("w4p", [12, 720], bf16, isOutput=False)
    w5t_d = nc.declare_dram_parameter("w5t", [86, 200], bf16, isOutput=False)
    # w6 packed: cols 0:676 = k-chunk a (rows 0:100 = w6[:, :100].T),
    # cols 676:1352 = k-chunk b (rows 0:100 = w6[:, 100:200].T, row 100 = b6)
    w6_d = nc.declare_dram_parameter("w6", [106, 1352], bf16, isOutput=False)
    out_d = nc.declare_dram_parameter("out", [13, NS * 168], f32, isOutput=True)

    es = ExitStack()

    def sb(name, shape, dt=bf16):
        return es.enter_context(nc.sbuf_tensor(name, shape, dt))

    def pt(name, shape):
        return es.enter_context(nc.psum_tensor(name, shape, f32))

    with es:
        x_t = sb("x_t", [64, NS * 168])
        wb_t = sb("wb_t", [64, 108])
        w4p_t = sb("w4p_t", [12, 720])
        w5t_t = sb("w5t_t", [86, 200])
        w6_t = sb("w6_t", [106, 1352])
        h1_t = sb("h1_t", [25, NS * 72])   # row 24 = ones (b2 rides w2b row 24)
        h2_t = sb("h2_t", [6, NS * 36])
        tmp6_t = sb("tmp6_t", [6, NS * 18])
        pool_t = sb("pool_t", [7, NS * 9])  # row 6 = ones (b4 rides w4p row 6)
        h4_t = sb("h4_t", [81, NS])         # row 80 = ones (b5 rides w5t row 80)
        t01 = sb("t01", [101, 2 * NS])      # cols 0:2 = t0, 2:4 = t1; row 100 = ones
        h6_t = sb("h6_t", [52, 13 * NS])
        out_t = sb("out_t", [13, NS * 168], f32)
        scr_t = sb("scr_t", [1, 2])         # bf16: table-preload dummy output
        zb_t = sb("zb_t", [101, 1], f32)    # zero bias for all sigmoids

        psum1 = pt("psum1", [24, NS * 72])
        psum2 = pt("psum2", [6, NS * 36])
        psum4 = pt("psum4", [80, NS])
        psum5 = pt("psum5", [100, 2 * NS])
        psum6 = pt("psum6", [52, 13 * NS])
        psume = pt("psume", [13, NS * 168])
        psum_scr = pt("psum_scr", [1, 2])

        dsA = es.enter_context(nc.semaphore("dsA"))    # wb (SP) + x (ACT)
        dsE = es.enter_context(nc.semaphore("dsE"))    # w4p (SWDGE)
        dsF = es.enter_context(nc.semaphore("dsF"))    # w5t (SWDGE)
        dsG = es.enter_context(nc.semaphore("dsG"))    # w6 row pieces x5 (SP ring)
        dsGs = es.enter_context(nc.semaphore("dsGs"))  # w6 last piece (SWDGE)
        dsO = es.enter_context(nc.semaphore("dsO"))    # output stores (no waiter)
        ssem = es.enter_context(nc.semaphore("ssem"))  # Pool ones-row memsets
        ssev = es.enter_context(nc.semaphore("ssev"))  # DVE zb/psum_scr memsets
        psem = es.enter_context(nc.semaphore("psem"))
        asem = es.enter_context(nc.semaphore("asem"))
        vsem = es.enter_context(nc.semaphore("vsem"))

        with nc.Block() as block:
            hoist = nc._hoist_insts = []

            @block.sync
            def _(sync):
                # wb first (tiny, conv1's stationary), then five w6 row-range
                # pieces. DMA engine assignment (measured): CONTIGUOUS
                # transfers <64KB round-robin 4-row batches across all 16
                # engines; strided (column-sliced) ones get a 2-engine path;
                # contiguous >=64KB pin to a single engine. Full-width
                # 18-row pieces (48.7KB) hit the fast path.
                hoist.append(sync.dma_start(out=wb_t[:], in_=wb_d[:]).then_inc(dsA, 16))
                hoist.append(sync.dma_start(out=w5t_t[:], in_=w5t_d[:]).then_inc(dsF, 16))
                for r0 in range(0, 90, 18):
                    hoist.append(
                        sync.dma_start(
                            out=w6_t[r0 : r0 + 18, :], in_=w6_d[r0 : r0 + 18, :]
                        ).then_inc(dsG, 16)
                    )
                # single full-width output store once both ACT copies land
                # (wait attached to the trigger: saves the standalone
                # EVENT_SEMAPHORE dispatch on the critical path)
                sync.dma_start(out=out_d[:, :], in_=out_t[:])._wait_ge(asem, 8).then_inc(dsO, 16)

            @block.vector
            def _(vector):
                # zb + psum_scr memsets gate only the ACT table preload
                hoist.append(vector.memset(psum_scr[:], 0.0).then_inc(ssev))
                hoist.append(vector.memset(zb_t[:], 0.0).then_inc(ssev))
                # (h, w) pooling as ONE 4-tap reduce over the (dh, dw) dims
                vector.wait_ge(ssem, 4)
                h2r = h2_t[:].rearrange(
                    "p (s hp dh wp dw) -> p (s hp) wp dh dw", s=NS, hp=3, dh=2, wp=3, dw=2
                )
                poolr = pool_t[0:6, :].rearrange("p (s hp wp) -> p (s hp) wp", s=NS, hp=3, wp=3)
                with nc.allow_low_precision("4-term bf16 pooling sum, matches prior impl"):
                    vector.tensor_reduce(
                        poolr[:], h2r[:], axis=mybir.AxisListType.XY, op=mybir.AluOpType.add
                    )._wait_ge(asem, 2).then_inc(vsem)  # 1


            @block.gpsimd
            def _(gpsimd):
                # ones memsets first (sig1 waits ssem at ~3.4us), then w4p
                # (needed ~5.5us) and the last w6 piece (needed ~7.4us).
                # Whole-tensor memsets: APs must start at partition 0;
                # writers of the non-ones rows wait on ssem first.
                hoist.append(gpsimd.memset(h1_t[:], 1.0).then_inc(ssem))
                hoist.append(gpsimd.memset(pool_t[:], 1.0).then_inc(ssem))
                hoist.append(gpsimd.memset(h4_t[:], 1.0).then_inc(ssem))
                hoist.append(gpsimd.memset(t01[:], 1.0).then_inc(ssem))
                hoist.append(gpsimd.dma_start(out=w4p_t[:], in_=w4p_d[:]).then_inc(dsE, 16))
                hoist.append(
                    gpsimd.dma_start(out=w6_t[90:106, :], in_=w6_d[90:106, :]).then_inc(dsGs, 16)
                )

            @block.scalar
            def _(scalar):
                # x rides the ACT ring alone: the ACT engine starts ~260ns
                # before SP, so conv1's gate lands earliest here, and the
                # table-load dummy follows with no trigger in its way.
                hoist.append(scalar.dma_start(out=x_t[:], in_=x_d[:]).then_inc(dsA, 16))
                # dummy sigmoid FIRST IN THIS BASIC BLOCK: walrus tracks ACT
                # tables per-bb, so the preload must live in the same bb as
                # the real sigmoids to avoid a 1.3us reload before sig1.
                # Stage gates ride as attached waits (one per instruction).
                scalar.activation(scr_t[:], psum_scr[:], Sig, bias=zb_t[0:1, :])._wait_ge(ssev, 2)
                scalar.wait_ge(ssem, 4)
                scalar.activation(h1_t[0:24, :], psum1[:], Sig, bias=zb_t[0:24, :])._wait_ge(psem, 1).then_inc(asem)  # 1
                scalar.activation(h2_t[:], psum2[:], Sig, bias=zb_t[0:6, :])._wait_ge(psem, 2).then_inc(asem)  # 2
                scalar.activation(h4_t[0:80, :], psum4[:], Sig, bias=zb_t[0:80, :])._wait_ge(psem, 3).then_inc(asem)  # 3
                scalar.activation(
                    t01[0:100, 0:NS], psum5[:, 0:NS], Sig, bias=zb_t[0:100, :]
                )._wait_ge(psem, 4).then_inc(asem)  # 4
                scalar.activation(
                    t01[0:100, NS : 2 * NS], psum5[:, NS : 2 * NS], Sig, bias=zb_t[0:100, :]
                )._wait_ge(psem, 5).then_inc(asem)  # 5
                scalar.activation(h6_t[:], psum6[:], Sig, bias=zb_t[0:52, :])._wait_ge(psem, 6).then_inc(asem)  # 6
                scalar.copy(out_t[:, 0:168], psume[:, 0:168])._wait_ge(psem, 7).then_inc(asem)  # 7
                scalar.copy(out_t[:, 168:336], psume[:, 168:336])._wait_ge(psem, 8).then_inc(asem)  # 8

            @block.tensor
            def _(tensor):
                # conv1: 4 accumulated matmuls; K=53 incl. the b1 ones-row.
                # The wb+x gate rides as an attached wait on the first matmul.
                x4 = x_t[0:53, :].rearrange("p (s h w) -> p s h w", s=NS, h=7, w=24)
                taps1 = [(kh, kw) for kh in range(2) for kw in range(2)]
                for k, (kh, kw) in enumerate(taps1):
                    mm = tensor.matmul(
                        psum1[:],
                        wb_t[0:53, k * 24 : (k + 1) * 24],
                        x4[:, :, kh : kh + 6, kw : kw + 23 : 2],
                        start=(k == 0),
                        stop=(k == 3),
                    )
                    if k == 0:
                        mm._wait_ge(dsA, 2 * _DMA_CREDITS)  # wb (SP) + x (ACT)
                    if k == 3:
                        mm.then_inc(psem)  # psem 1
                # conv2: K=25 incl. the b2 ones-row. asem>=1 transitively
                # covers the Pool ones memsets (ACT waits ssem before sig1).
                h14 = h1_t[:].rearrange("p (s h w) -> p s h w", s=NS, h=6, w=12)
                for kw in range(2):
                    mm = tensor.matmul(
                        psum2[:],
                        wb_t[0:25, 96 + kw * 6 : 96 + (kw + 1) * 6],
                        h14[:, :, :, kw : kw + 11 : 2],
                        start=(kw == 0),
                        stop=(kw == 1),
                    )
                    if kw == 0:
                        mm._wait_ge(asem, 1)
                    if kw == 1:
                        mm.then_inc(psem)  # psem 2
                # early-satisfied DMA gates, placed off the hot handoffs
                tensor.wait_ge(dsF, 16)
                tensor.wait_ge(dsE, 16)
                # fc4: 9 (hp,wp) matmuls vs the h/w-pooled tile; d-pooling and
                # /8 live in w4p; j=0 has K=7 incl. the b4 ones-row
                pool4 = pool_t[:].rearrange("p (s j) -> p s j", s=NS, j=9)
                for j in range(9):
                    kk = 7 if j == 0 else 6
                    mm = tensor.matmul(
                        psum4[:],
                        w4p_t[0:kk, j * 80 : (j + 1) * 80],
                        pool4[0:kk, :, j],
                        start=(j == 0),
                        stop=(j == 8),
                    )
                    if j == 0:
                        mm._wait_ge(vsem, 1)
                    if j == 8:
                        mm.then_inc(psem)  # psem 3
                # fc5: two k-halves, each gating its own sigmoid so fc6's
                # a-matmuls can start one ACT op earlier
                tensor.matmul(
                    psum5[:, 0:NS], w5t_t[0:81, 0:100], h4_t[:], start=True, stop=True
                )._wait_ge(asem, 3).then_inc(psem)  # psem 4
                tensor.matmul(
                    psum5[:, NS : 2 * NS], w5t_t[0:81, 100:200], h4_t[:], start=True, stop=True
                ).then_inc(psem)  # psem 5
                # fc6: 13 i-chunks x 2 k-chunks (a_i, b_i interleaved — only
                # one PSUM accumulation group may be open per zero region)
                tensor.wait_ge(dsG, 80)
                tensor.wait_ge(dsGs, 16)
                for i in range(13):
                    mma = tensor.matmul(
                        psum6[:, i * NS : (i + 1) * NS],
                        w6_t[0:100, i * 52 : (i + 1) * 52],
                        t01[0:100, 0:NS],
                        start=True,
                        stop=False,
                    )
                    if i == 0:
                        mma._wait_ge(asem, 5)
                    mm = tensor.matmul(
                        psum6[:, i * NS : (i + 1) * NS],
                        w6_t[0:101, 676 + i * 52 : 676 + (i + 1) * 52],
                        t01[:, NS : 2 * NS],
                        start=False,
                        stop=True,
                    )
                    if i == 12:
                        mm.then_inc(psem)  # psem 6
                # einsum; each sample region releases its own copy engine
                h6v = h6_t[:].rearrange("p (i s) -> p s i", s=NS)
                for s in range(NS):
                    mm = tensor.matmul(
                        psume[:, s * 168 : (s + 1) * 168],
                        h6v[:, s, :],
                        x_t[0:52, s * 168 : (s + 1) * 168],
                        start=True,
                        stop=True,
                    ).then_inc(psem)  # psem 7, 8
                    if s == 0:
                        mm._wait_ge(asem, 6)

    _strip_entry_barrier(nc)
    return nc


def _strip_entry_barrier(nc):
    f = nc.m.functions[0]
    bbs = {bb.name: bb for bb in f.blocks}
    main = bbs["main"]
    # 1) drop the init all-engine barrier (nothing reads the const-AP tiles)
    main.instructions = [
        i
        for i in main.instructions
        if not (
            i.name.startswith("barrier_")
            or getattr(i, "opcode", "") == "Drain"
            or type(i).__name__ == "InstDrain"
        )
    ]
    # 2) hoist the input-DMA triggers into main so transfers start during the
    #    preamble, before the Block-entry rendezvous
    hoisted = {bi.ins.name for bi in getattr(nc, "_hoist_insts", [])}
    if hoisted:
        moved = []
        for bb in f.blocks:
            if bb.name == "main" or not bb.instructions:
                continue
            keep = []
            for i in bb.instructions:
                (moved if i.name in hoisted else keep).append(i)
            if len(keep) != len(bb.instructions):
                bb.instructions = keep
        # insert at the very top of main (after the entry Call): the DMA
        # triggers use only immediates + the parameter table, not the
        # preamble registers
        insts = main.instructions
        main.instructions = insts[:1] + moved + insts[1:]


def _prep_weights(w1, b1, w2, b2, w4, b4, w5, b5, w6, b6):
    f = np.float32
    w1v = np.asarray(w1, f)[0, 0]  # (6,2,2)
    w2v = np.asarray(w2, f)[0, 0, :, 0, :]  # (4,2)
    w4 = np.asarray(w4, f)
    w5 = np.asarray(w5, f)
    w6 = np.asarray(w6, f)
    b1 = np.asarray(b1, f)
    b2 = np.asarray(b2, f)
    b4 = np.asarray(b4, f)
    b5 = np.asarray(b5, f)
    b6 = np.asarray(b6, f)

    wb = np.zeros((64, 108), f)
    for kd in range(6):
        for kh in range(2):
            for kw in range(2):
                for d in range(24):
                    wb[2 * d + kd, (kh * 2 + kw) * 24 + d] = w1v[kd, kh, kw]
    wb[52, 0:24] = b1[0]  # ones-row bias, k=0 tap block only
    for kd in range(4):
        for kw in range(2):
            for d in range(6):
                wb[4 * d + kd, 96 + kw * 6 + d] = w2v[kd, kw]
    wb[24, 96:102] = b2[0]  # ones-row bias, kw=0 block only

    w4r = w4.reshape(80, 3, 3, 3) / 8.0
    w4q = np.transpose(w4r, (1, 2, 3, 0)).reshape(3, 720)
    w4p = np.zeros((12, 720), f)
    w4p[0:6:2, :] = w4q
    w4p[1:6:2, :] = w4q
    w4p[6, 0:80] = b4  # ones-row bias, j=0 block only

    w5t = np.zeros((86, 200), f)
    w5t[0:80, :] = w5.T
    w5t[80, :] = b5

    w6p = np.zeros((106, 1352), f)
    w6p[0:100, 0:676] = w6[:, 0:100].T
    w6p[0:100, 676:1352] = w6[:, 100:200].T
    w6p[100, 676:1352] = b6

    return dict(
        wb=wb.astype(BF16),
        w4p=w4p.astype(BF16),
        w5t=w5t.astype(BF16),
        w6=w6p.astype(BF16),
    )


def kernel(x, w1, b1, w2, b2, w4, b4, w5, b5, w6, b6, _trace=False):
    global LAST_EXEC_NS, LAST_RESULT
    from concourse.bass_utils import run_bass_kernel_spmd

    if "nc" not in _BUILT:
        _BUILT["nc"] = _build_nc()
    nc = _BUILT["nc"]

    xs = np.ascontiguousarray(np.asarray(x, np.float32).reshape(10, 52, 168))
    wd = _prep_weights(w1, b1, w2, b2, w4, b4, w5, b5, w6, b6)

    in_maps = []
    for i in range(N_CORES):
        xc = np.ones((64, NS * 168), np.float32)
        xc[0:52] = np.transpose(np.stack([xs[a] for a in ASSIGN[i]]), (1, 0, 2)).reshape(52, NS * 168)
        xc = np.ascontiguousarray(xc.astype(BF16))
        m = {"x": xc}
        m.update(wd)
        in_maps.append(m)

    res = run_bass_kernel_spmd(nc, in_maps, core_ids=list(range(N_CORES)), trace=_trace)
    LAST_EXEC_NS = res.exec_time_ns
    LAST_RESULT = res

    out = np.zeros((10, 2184), np.float32)
    for i in range(N_CORES):
        o = res.results[i]["out"].reshape(13, NS, 168)
        for slot, b in enumerate(ASSIGN[i]):
            out[b] = o[:, slot, :].reshape(2184)
    return out


# revision 35
# speedup vs baseline: 1.2206x; 1.0205x over previous
"""Trainium2 Bass kernel for nn_C3DNet — data-parallel over the 10 samples on 8 cores.

Math (per sample, from the reference):
  x:(52,7,24) -conv1(6,2,2)s(2,1,2)+sig-> (24,6,12) -conv2(4,1,2)s(4,1,2)+sig-> (6,6,6)
  -avgpool2-> 27 -fc4+sig-> 80 -fc5+sig-> 200 -fc6+sig-> 676
  out = h6.reshape(13,52) @ x.reshape(52,168)  -> (13,168) -> 2184

Everything is cast as TensorE matmuls (bf16 datapath, f32 PSUM):
  * conv1/conv2 contract the D dimension (on partitions) using host-built
    banded weight matrices; the (h,w) taps become strided free-dim views.
  * fc4 contracts q=3 partitions x 9 (hp,wp) matmuls; b1/b2/b4 applied via the
    ACT sigmoid's per-partition bias operand; b5/b6 folded via ones-rows.
  * fc6 emits PSUM [52, (i,s)] directly so the final einsum lhsT needs no
    transpose; its 26 matmuls are split into a 13-a (k-chunk 0) + 13-b
    (k-chunk 1) sequence gated by per-half fc5 sigmoids so the a-half starts
    one sigmoid earlier.

Schedule notes (v2, from the v1 perfetto trace):
  * w6 (286KB bf16) was the body bottleneck in v1: whole-row-contiguous DMAs
    were each pinned to a single DMA engine (~22GB/s), landing at ~16us and
    gating fc6. v2 ships w6 as four 338-column chunks (676B rows, the same
    row size as x, which provably round-robins across all 16 DMA engines)
    spread over the SP and DVE HWDGE rings, all triggered in the preamble.
  * The Activation ring issues NO DMAs: ACT does the sigmoid-table preload
    dummy immediately (gated only on the DVE zb/psum_scr memsets), then is
    free for the sigmoid chain.
  * Pool memsets touch ONLY the ones-rows (disjoint from the sigmoid output
    rows), removing the write-order hazard; PE waits ssem once before conv2.
  * Output: einsum region s is copied PSUM->SBUF by ACT (s=0) / DVE (s=1) in
    parallel, and each region gets its own DRAM store (SP ring / DVE ring).

Raw-bass (Block + explicit semaphores): this walrus build only supports ONE
attached sync-wait per Matmult/DMA instruction, so standalone wait_ge
instructions are used. DMA completion order is not guaranteed across queues,
so consumers wait for the issuing group's FULL credit count (16 per DMA).
"""

import sys
from contextlib import ExitStack

sys.path.insert(0, "/opt/trn_rl_repo")

import numpy as np
import ml_dtypes

# Each DMA delivers 16 completion credits; waiting below 16 proved
# nondeterministic on this runtime, so all consumers wait for the full count.
_DMA_CREDITS = 16

BF16 = ml_dtypes.bfloat16

N_CORES = 8
NS = 2  # sample slots per core
# core i handles samples ASSIGN[i]; host gathers accordingly
ASSIGN = [[0, 8], [1, 9]] + [[i, i] for i in range(2, N_CORES)]

LAST_EXEC_NS = None
LAST_RESULT = None

_BUILT = {}


def _build_nc():
    import concourse.bass as bass
    import concourse.mybir as mybir

    f32 = mybir.dt.float32
    bf16 = mybir.dt.bfloat16
    Sig = mybir.ActivationFunctionType.Sigmoid

    nc = bass.Bass()

    # x rows 0:52 = sample data, row 52 = ones (carries b1 via wb row 52)
    x_d = nc.declare_dram_parameter("x", [64, NS * 168], bf16, isOutput=False)
    # wb: w1b (96 cols, rows 0:53 incl. b1 ones-row) ++ w2b (12, rows 0:25
    # incl. b2 ones-row)
    wb_d = nc.declare_dram_parameter("wb", [64, 108], bf16, isOutput=False)
    # w4p row 6 = b4 in the j=0 block, zeros elsewhere
    w4p_d = nc.declare_dram_parameter("w4p", [12, 720], bf16, isOutput=False)
    w5t_d = nc.declare_dram_parameter("w5t", [86, 200], bf16, isOutput=False)
    # w6 packed: cols 0:676 = k-chunk a (rows 0:100 = w6[:, :100].T),
    # cols 676:1352 = k-chunk b (rows 0:100 = w6[:, 100:200].T, row 100 = b6)
    w6_d = nc.declare_dram_parameter("w6", [106, 1352], bf16, isOutput=False)
    out_d = nc.declare_dram_parameter("out", [13, NS * 168], f32, isOutput=True)

    es = ExitStack()

    def sb(name, shape, dt=bf16):
        return es.enter_context(nc.sbuf_tensor(name, shape, dt))

    def pt(name, shape):
        return es.enter_context(nc.psum_tensor(name, shape, f32))

    with es:
        x_t = sb("x_t", [64, NS * 168])
        wb_t = sb("wb_t", [64, 108])
        w4p_t = sb("w4p_t", [12, 720])
        w5t_t = sb("w5t_t", [86, 200])
        w6_t = sb("w6_t", [106, 1352])
        h1_t = sb("h1_t", [25, NS * 72])   # row 24 = ones (b2 rides w2b row 24)
        h2_t = sb("h2_t", [6, NS * 36])
        tmp6_t = sb("tmp6_t", [6, NS * 18])
        pool_t = sb("pool_t", [7, NS * 9])  # row 6 = ones (b4 rides w4p row 6)
        h4_t = sb("h4_t", [81, NS])         # row 80 = ones (b5 rides w5t row 80)
        t01 = sb("t01", [101, 2 * NS])      # cols 0:2 = t0, 2:4 = t1; row 100 = ones
        h6_t = sb("h6_t", [52, 13 * NS])
        out_t = sb("out_t", [13, NS * 168], f32)
        scr_t = sb("scr_t", [1, 2])         # bf16: table-preload dummy output
        zb_t = sb("zb_t", [101, 1], f32)    # zero bias for all sigmoids

        psum1 = pt("psum1", [24, NS * 72])
        psum2 = pt("psum2", [6, NS * 36])
        psum4 = pt("psum4", [80, NS])
        psum5 = pt("psum5", [100, 2 * NS])
        psum6 = pt("psum6", [52, 13 * NS])
        psume = pt("psume", [13, NS * 168])
        psum_scr = pt("psum_scr", [1, 2])

        dsA = es.enter_context(nc.semaphore("dsA"))    # wb (SP) + x (ACT)
        dsE = es.enter_context(nc.semaphore("dsE"))    # w4p (SWDGE)
        dsF = es.enter_context(nc.semaphore("dsF"))    # w5t (SWDGE)
        dsG = es.enter_context(nc.semaphore("dsG"))    # w6 row pieces x5 (SP ring)
        dsGs = es.enter_context(nc.semaphore("dsGs"))  # w6 last piece (SWDGE)
        dsO = es.enter_context(nc.semaphore("dsO"))    # output stores (no waiter)
        ssem = es.enter_context(nc.semaphore("ssem"))  # Pool ones-row memsets
        ssev = es.enter_context(nc.semaphore("ssev"))  # DVE zb/psum_scr memsets
        psem = es.enter_context(nc.semaphore("psem"))
        asem = es.enter_context(nc.semaphore("asem"))
        vsem = es.enter_context(nc.semaphore("vsem"))

        with nc.Block() as block:
            hoist = nc._hoist_insts = []

            @block.sync
            def _(sync):
                # wb first (tiny, conv1's stationary), then five w6 row-range
                # pieces. DMA engine assignment (measured): CONTIGUOUS
                # transfers <64KB round-robin 4-row batches across all 16
                # engines; strided (column-sliced) ones get a 2-engine path;
                # contiguous >=64KB pin to a single engine. Full-width
                # 18-row pieces (48.7KB) hit the fast path.
                hoist.append(sync.dma_start(out=wb_t[:], in_=wb_d[:]).then_inc(dsA, 16))
                for r0 in range(0, 88, 22):
                    hoist.append(
                        sync.dma_start(
                            out=w6_t[r0 : r0 + 22, :], in_=w6_d[r0 : r0 + 22, :]
                        ).then_inc(dsG, 16)
                    )
                # single full-width output store once both ACT copies land
                # (wait attached to the trigger: saves the standalone
                # EVENT_SEMAPHORE dispatch on the critical path)
                sync.dma_start(out=out_d[:, :], in_=out_t[:])._wait_ge(asem, 8).then_inc(dsO, 16)

            @block.vector
            def _(vector):
                # zb + psum_scr memsets gate only the ACT table preload
                hoist.append(vector.memset(psum_scr[:], 0.0).then_inc(ssev))
                hoist.append(vector.memset(zb_t[:], 0.0).then_inc(ssev))
                # (h, w) pooling as ONE 4-tap reduce over the (dh, dw) dims
                vector.wait_ge(ssem, 4)
                h2r = h2_t[:].rearrange(
                    "p (s hp dh wp dw) -> p (s hp) wp dh dw", s=NS, hp=3, dh=2, wp=3, dw=2
                )
                poolr = pool_t[0:6, :].rearrange("p (s hp wp) -> p (s hp) wp", s=NS, hp=3, wp=3)
                with nc.allow_low_precision("4-term bf16 pooling sum, matches prior impl"):
                    vector.tensor_reduce(
                        poolr[:], h2r[:], axis=mybir.AxisListType.XY, op=mybir.AluOpType.add
                    )._wait_ge(asem, 2).then_inc(vsem)  # 1


            @block.gpsimd
            def _(gpsimd):
                # ones memsets first (sig1 waits ssem at ~3.4us), then w4p
                # (needed ~5.5us) and the last w6 piece (needed ~7.4us).
                # Whole-tensor memsets: APs must start at partition 0;
                # writers of the non-ones rows wait on ssem first.
                hoist.append(gpsimd.memset(h1_t[:], 1.0).then_inc(ssem))
                hoist.append(gpsimd.memset(pool_t[:], 1.0).then_inc(ssem))
                hoist.append(gpsimd.memset(h4_t[:], 1.0).then_inc(ssem))
                hoist.append(gpsimd.memset(t01[:], 1.0).then_inc(ssem))
                hoist.append(gpsimd.dma_start(out=w4p_t[:], in_=w4p_d[:]).then_inc(dsE, 16))
                hoist.append(gpsimd.dma_start(out=w5t_t[:], in_=w5t_d[:]).then_inc(dsF, 16))
                hoist.append(
                    gpsimd.dma_start(out=w6_t[88:106, :], in_=w6_d[88:106, :]).then_inc(dsGs, 16)
                )

            @block.scalar
            def _(scalar):
                # x rides the ACT ring alone: the ACT engine starts ~260ns
                # before SP, so conv1's gate lands earliest here, and the
                # table-load dummy follows with no trigger in its way.
                hoist.append(scalar.dma_start(out=x_t[:], in_=x_d[:]).then_inc(dsA, 16))
                # dummy sigmoid FIRST IN THIS BASIC BLOCK: walrus tracks ACT
                # tables per-bb, so the preload must live in the same bb as
                # the real sigmoids to avoid a 1.3us reload before sig1.
                # Stage gates ride as attached waits (one per instruction).
                scalar.activation(scr_t[:], psum_scr[:], Sig, bias=zb_t[0:1, :])._wait_ge(ssev, 2)
                scalar.wait_ge(ssem, 4)
                scalar.activation(h1_t[0:24, :], psum1[:], Sig, bias=zb_t[0:24, :])._wait_ge(psem, 1).then_inc(asem)  # 1
                scalar.activation(h2_t[:], psum2[:], Sig, bias=zb_t[0:6, :])._wait_ge(psem, 2).then_inc(asem)  # 2
                scalar.activation(h4_t[0:80, :], psum4[:], Sig, bias=zb_t[0:80, :])._wait_ge(psem, 3).then_inc(asem)  # 3
                scalar.activation(
                    t01[0:100, 0:NS], psum5[:, 0:NS], Sig, bias=zb_t[0:100, :]
                )._wait_ge(psem, 4).then_inc(asem)  # 4
                scalar.activation(
                    t01[0:100, NS : 2 * NS], psum5[:, NS : 2 * NS], Sig, bias=zb_t[0:100, :]
                )._wait_ge(psem, 5).then_inc(asem)  # 5
                scalar.activation(h6_t[:], psum6[:], Sig, bias=zb_t[0:52, :])._wait_ge(psem, 6).then_inc(asem)  # 6
                scalar.copy(out_t[:, 0:168], psume[:, 0:168])._wait_ge(psem, 7).then_inc(asem)  # 7
                scalar.copy(out_t[:, 168:336], psume[:, 168:336])._wait_ge(psem, 8).then_inc(asem)  # 8

            @block.tensor
            def _(tensor):
                # conv1: 4 accumulated matmuls; K=53 incl. the b1 ones-row.
                # The wb+x gate rides as an attached wait on the first matmul.
                x4 = x_t[0:53, :].rearrange("p (s h w) -> p s h w", s=NS, h=7, w=24)
                taps1 = [(kh, kw) for kh in range(2) for kw in range(2)]
                for k, (kh, kw) in enumerate(taps1):
                    mm = tensor.matmul(
                        psum1[:],
                        wb_t[0:53, k * 24 : (k + 1) * 24],
                        x4[:, :, kh : kh + 6, kw : kw + 23 : 2],
                        start=(k == 0),
                        stop=(k == 3),
                    )
                    if k == 0:
                        mm._wait_ge(dsA, 2 * _DMA_CREDITS)  # wb (SP) + x (ACT)
                    if k == 3:
                        mm.then_inc(psem)  # psem 1
                # conv2: K=25 incl. the b2 ones-row. asem>=1 transitively
                # covers the Pool ones memsets (ACT waits ssem before sig1).
                h14 = h1_t[:].rearrange("p (s h w) -> p s h w", s=NS, h=6, w=12)
                for kw in range(2):
                    mm = tensor.matmul(
                        psum2[:],
                        wb_t[0:25, 96 + kw * 6 : 96 + (kw + 1) * 6],
                        h14[:, :, :, kw : kw + 11 : 2],
                        start=(kw == 0),
                        stop=(kw == 1),
                    )
                    if kw == 0:
                        mm._wait_ge(asem, 1)
                    if kw == 1:
                        mm.then_inc(psem)  # psem 2
                # early-satisfied DMA gates, placed off the hot handoffs
                tensor.wait_ge(dsF, 16)
                tensor.wait_ge(dsE, 16)
                # fc4: 9 (hp,wp) matmuls vs the h/w-pooled tile; d-pooling and
                # /8 live in w4p; j=0 has K=7 incl. the b4 ones-row
                pool4 = pool_t[:].rearrange("p (s j) -> p s j", s=NS, j=9)
                for j in range(9):
                    kk = 7 if j == 0 else 6
                    mm = tensor.matmul(
                        psum4[:],
                        w4p_t[0:kk, j * 80 : (j + 1) * 80],
                        pool4[0:kk, :, j],
                        start=(j == 0),
                        stop=(j == 8),
                    )
                    if j == 0:
                        mm._wait_ge(vsem, 1)
                    if j == 8:
                        mm.then_inc(psem)  # psem 3
                # fc5: two k-halves, each gating its own sigmoid so fc6's
                # a-matmuls can start one ACT op earlier
                tensor.matmul(
                    psum5[:, 0:NS], w5t_t[0:81, 0:100], h4_t[:], start=True, stop=True
                )._wait_ge(asem, 3).then_inc(psem)  # psem 4
                tensor.matmul(
                    psum5[:, NS : 2 * NS], w5t_t[0:81, 100:200], h4_t[:], start=True, stop=True
                ).then_inc(psem)  # psem 5
                # fc6: 13 i-chunks x 2 k-chunks (a_i, b_i interleaved — only
                # one PSUM accumulation group may be open per zero region)
                tensor.wait_ge(dsG, 64)
                tensor.wait_ge(dsGs, 16)
                for i in range(13):
                    mma = tensor.matmul(
                        psum6[:, i * NS : (i + 1) * NS],
                        w6_t[0:100, i * 52 : (i + 1) * 52],
                        t01[0:100, 0:NS],
                        start=True,
                        stop=False,
                    )
                    if i == 0:
                        mma._wait_ge(asem, 5)
                    mm = tensor.matmul(
                        psum6[:, i * NS : (i + 1) * NS],
                        w6_t[0:101, 676 + i * 52 : 676 + (i + 1) * 52],
                        t01[:, NS : 2 * NS],
                        start=False,
                        stop=True,
                    )
                    if i == 12:
                        mm.then_inc(psem)  # psem 6
                # einsum; each sample region releases its own copy engine
                h6v = h6_t[:].rearrange("p (i s) -> p s i", s=NS)
                for s in range(NS):
                    mm = tensor.matmul(
                        psume[:, s * 168 : (s + 1) * 168],
                        h6v[:, s, :],
                        x_t[0:52, s * 168 : (s + 1) * 168],
                        start=True,
                        stop=True,
                    ).then_inc(psem)  # psem 7, 8
                    if s == 0:
                        mm._wait_ge(asem, 6)

    _strip_entry_barrier(nc)
    return nc


def _strip_entry_barrier(nc):
    f = nc.m.functions[0]
    bbs = {bb.name: bb for bb in f.blocks}
    main = bbs["main"]
    # 1) drop the init all-engine barrier (nothing reads the const-AP tiles)
    main.instructions = [
        i
        for i in main.instructions
        if not (
            i.name.startswith("barrier_")
            or getattr(i, "opcode", "") == "Drain"
            or type(i).__name__ == "InstDrain"
        )
    ]
    # 2) hoist the input-DMA triggers into main so transfers start during the
    #    preamble, before the Block-entry rendezvous
    hoisted = {bi.ins.name for bi in getattr(nc, "_hoist_insts", [])}
    if hoisted:
        moved = []
        for bb in f.blocks:
            if bb.name == "main" or not bb.instructions:
                continue
            keep = []
            for i in bb.instructions:
                (moved if i.name in hoisted else keep).append(i)
            if len(keep) != len(bb.instructions):
                bb.instructions = keep
        # insert at the very top of main (after the entry Call): the DMA
        # triggers use only immediates + the parameter table, not the
        # preamble registers
        insts = main.instructions
        main.instructions = insts[:1] + moved + insts[1:]


def _prep_weights(w1, b1, w2, b2, w4, b4, w5, b5, w6, b6):
    f = np.float32
    w1v = np.asarray(w1, f)[0, 0]  # (6,2,2)
    w2v = np.asarray(w2, f)[0, 0, :, 0, :]  # (4,2)
    w4 = np.asarray(w4, f)
    w5 = np.asarray(w5, f)
    w6 = np.asarray(w6, f)
    b1 = np.asarray(b1, f)
    b2 = np.asarray(b2, f)
    b4 = np.asarray(b4, f)
    b5 = np.asarray(b5, f)
    b6 = np.asarray(b6, f)

    wb = np.zeros((64, 108), f)
    for kd in range(6):
        for kh in range(2):
            for kw in range(2):
                for d in range(24):
                    wb[2 * d + kd, (kh * 2 + kw) * 24 + d] = w1v[kd, kh, kw]
    wb[52, 0:24] = b1[0]  # ones-row bias, k=0 tap block only
    for kd in range(4):
        for kw in range(2):
            for d in range(6):
                wb[4 * d + kd, 96 + kw * 6 + d] = w2v[kd, kw]
    wb[24, 96:102] = b2[0]  # ones-row bias, kw=0 block only

    w4r = w4.reshape(80, 3, 3, 3) / 8.0
    w4q = np.transpose(w4r, (1, 2, 3, 0)).reshape(3, 720)
    w4p = np.zeros((12, 720), f)
    w4p[0:6:2, :] = w4q
    w4p[1:6:2, :] = w4q
    w4p[6, 0:80] = b4  # ones-row bias, j=0 block only

    w5t = np.zeros((86, 200), f)
    w5t[0:80, :] = w5.T
    w5t[80, :] = b5

    w6p = np.zeros((106, 1352), f)
    w6p[0:100, 0:676] = w6[:, 0:100].T
    w6p[0:100, 676:1352] = w6[:, 100:200].T
    w6p[100, 676:1352] = b6

    return dict(
        wb=wb.astype(BF16),
        w4p=w4p.astype(BF16),
        w5t=w5t.astype(BF16),
        w6=w6p.astype(BF16),
    )


def kernel(x, w1, b1, w2, b2, w4, b4, w5, b5, w6, b6, _trace=False):
    global LAST_EXEC_NS, LAST_RESULT
    from concourse.bass_utils import run_bass_kernel_spmd

    if "nc" not in _BUILT:
        _BUILT["nc"] = _build_nc()
    nc = _BUILT["nc"]

    xs = np.ascontiguousarray(np.asarray(x, np.float32).reshape(10, 52, 168))
    wd = _prep_weights(w1, b1, w2, b2, w4, b4, w5, b5, w6, b6)

    in_maps = []
    for i in range(N_CORES):
        xc = np.ones((64, NS * 168), np.float32)
        xc[0:52] = np.transpose(np.stack([xs[a] for a in ASSIGN[i]]), (1, 0, 2)).reshape(52, NS * 168)
        xc = np.ascontiguousarray(xc.astype(BF16))
        m = {"x": xc}
        m.update(wd)
        in_maps.append(m)

    res = run_bass_kernel_spmd(nc, in_maps, core_ids=list(range(N_CORES)), trace=_trace)
    LAST_EXEC_NS = res.exec_time_ns
    LAST_RESULT = res

    out = np.zeros((10, 2184), np.float32)
    for i in range(N_CORES):
        o = res.results[i]["out"].reshape(13, NS, 168)
        for slot, b in enumerate(ASSIGN[i]):
            out[b] = o[:, slot, :].reshape(2184)
    return out


# revision 36
# speedup vs baseline: 1.2661x; 1.0372x over previous
"""Trainium2 Bass kernel for nn_C3DNet — data-parallel over the 10 samples on 8 cores.

Math (per sample, from the reference):
  x:(52,7,24) -conv1(6,2,2)s(2,1,2)+sig-> (24,6,12) -conv2(4,1,2)s(4,1,2)+sig-> (6,6,6)
  -avgpool2-> 27 -fc4+sig-> 80 -fc5+sig-> 200 -fc6+sig-> 676
  out = h6.reshape(13,52) @ x.reshape(52,168)  -> (13,168) -> 2184

Everything is cast as TensorE matmuls (bf16 datapath, f32 PSUM):
  * conv1/conv2 contract the D dimension (on partitions) using host-built
    banded weight matrices; the (h,w) taps become strided free-dim views.
  * fc4 contracts q=3 partitions x 9 (hp,wp) matmuls; b1/b2/b4 applied via the
    ACT sigmoid's per-partition bias operand; b5/b6 folded via ones-rows.
  * fc6 emits PSUM [52, (i,s)] directly so the final einsum lhsT needs no
    transpose; its 26 matmuls are split into a 13-a (k-chunk 0) + 13-b
    (k-chunk 1) sequence gated by per-half fc5 sigmoids so the a-half starts
    one sigmoid earlier.

Schedule notes (v2, from the v1 perfetto trace):
  * w6 (286KB bf16) was the body bottleneck in v1: whole-row-contiguous DMAs
    were each pinned to a single DMA engine (~22GB/s), landing at ~16us and
    gating fc6. v2 ships w6 as four 338-column chunks (676B rows, the same
    row size as x, which provably round-robins across all 16 DMA engines)
    spread over the SP and DVE HWDGE rings, all triggered in the preamble.
  * The Activation ring issues NO DMAs: ACT does the sigmoid-table preload
    dummy immediately (gated only on the DVE zb/psum_scr memsets), then is
    free for the sigmoid chain.
  * Pool memsets touch ONLY the ones-rows (disjoint from the sigmoid output
    rows), removing the write-order hazard; PE waits ssem once before conv2.
  * Output: einsum region s is copied PSUM->SBUF by ACT (s=0) / DVE (s=1) in
    parallel, and each region gets its own DRAM store (SP ring / DVE ring).

Raw-bass (Block + explicit semaphores): this walrus build only supports ONE
attached sync-wait per Matmult/DMA instruction, so standalone wait_ge
instructions are used. DMA completion order is not guaranteed across queues,
so consumers wait for the issuing group's FULL credit count (16 per DMA).
"""

import sys
from contextlib import ExitStack

sys.path.insert(0, "/opt/trn_rl_repo")

import numpy as np
import ml_dtypes

# Each DMA delivers 16 completion credits; waiting below 16 proved
# nondeterministic on this runtime, so all consumers wait for the full count.
_DMA_CREDITS = 16

BF16 = ml_dtypes.bfloat16

N_CORES = 8
NS = 2  # sample slots per core
# core i handles samples ASSIGN[i]; host gathers accordingly
ASSIGN = [[0, 8], [1, 9]] + [[i, i] for i in range(2, N_CORES)]

LAST_EXEC_NS = None
LAST_RESULT = None

_BUILT = {}


def _build_nc():
    import concourse.bass as bass
    import concourse.mybir as mybir

    f32 = mybir.dt.float32
    bf16 = mybir.dt.bfloat16
    Sig = mybir.ActivationFunctionType.Sigmoid

    nc = bass.Bass()

    # x rows 0:52 = sample data, row 52 = ones (carries b1 via wb row 52)
    x_d = nc.declare_dram_parameter("x", [64, NS * 168], bf16, isOutput=False)
    # wb: w1b (96 cols, rows 0:53 incl. b1 ones-row) ++ w2b (12, rows 0:25
    # incl. b2 ones-row)
    wb_d = nc.declare_dram_parameter("wb", [64, 108], bf16, isOutput=False)
    # w4p row 6 = b4 in the j=0 block, zeros elsewhere
    w4p_d = nc.declare_dram_parameter("w4p", [12, 720], bf16, isOutput=False)
    w5t_d = nc.declare_dram_parameter("w5t", [86, 200], bf16, isOutput=False)
    # w6 packed: cols 0:676 = k-chunk a (rows 0:100 = w6[:, :100].T),
    # cols 676:1352 = k-chunk b (rows 0:100 = w6[:, 100:200].T, row 100 = b6)
    w6_d = nc.declare_dram_parameter("w6", [106, 1352], bf16, isOutput=False)
    out_d = nc.declare_dram_parameter("out", [13, NS * 168], f32, isOutput=True)

    es = ExitStack()

    def sb(name, shape, dt=bf16):
        return es.enter_context(nc.sbuf_tensor(name, shape, dt))

    def pt(name, shape):
        return es.enter_context(nc.psum_tensor(name, shape, f32))

    with es:
        x_t = sb("x_t", [64, NS * 168])
        wb_t = sb("wb_t", [64, 108])
        w4p_t = sb("w4p_t", [12, 720])
        w5t_t = sb("w5t_t", [86, 200])
        w6_t = sb("w6_t", [106, 1352])
        h1_t = sb("h1_t", [25, NS * 72])   # row 24 = ones (b2 rides w2b row 24)
        h2_t = sb("h2_t", [6, NS * 36])
        tmp6_t = sb("tmp6_t", [6, NS * 18])
        pool_t = sb("pool_t", [7, NS * 9])  # row 6 = ones (b4 rides w4p row 6)
        h4_t = sb("h4_t", [81, NS])         # row 80 = ones (b5 rides w5t row 80)
        t01 = sb("t01", [101, 2 * NS])      # cols 0:2 = t0, 2:4 = t1; row 100 = ones
        h6_t = sb("h6_t", [52, 13 * NS])
        out_t = sb("out_t", [13, NS * 168], f32)
        scr_t = sb("scr_t", [1, 2])         # bf16: table-preload dummy output
        zb_t = sb("zb_t", [101, 1], f32)    # zero bias for all sigmoids

        psum1 = pt("psum1", [24, NS * 72])
        psum2 = pt("psum2", [6, NS * 36])
        psum4 = pt("psum4", [80, NS])
        psum5 = pt("psum5", [100, 2 * NS])
        psum6 = pt("psum6", [52, 13 * NS])
        psume = pt("psume", [13, NS * 168])
        psum_scr = pt("psum_scr", [1, 2])

        dsA = es.enter_context(nc.semaphore("dsA"))    # wb (SP) + x (ACT)
        dsE = es.enter_context(nc.semaphore("dsE"))    # w4p (SWDGE)
        dsF = es.enter_context(nc.semaphore("dsF"))    # w5t (SWDGE)
        dsG = es.enter_context(nc.semaphore("dsG"))    # w6 row pieces x5 (SP ring)
        dsGs = es.enter_context(nc.semaphore("dsGs"))  # w6 last piece (SWDGE)
        dsO = es.enter_context(nc.semaphore("dsO"))    # output stores (no waiter)
        ssem = es.enter_context(nc.semaphore("ssem"))  # Pool ones-row memsets
        ssev = es.enter_context(nc.semaphore("ssev"))  # DVE zb/psum_scr memsets
        psem = es.enter_context(nc.semaphore("psem"))
        asem = es.enter_context(nc.semaphore("asem"))
        vsem = es.enter_context(nc.semaphore("vsem"))

        with nc.Block() as block:
            hoist = nc._hoist_insts = []

            @block.sync
            def _(sync):
                # wb first (tiny, conv1's stationary), then five w6 row-range
                # pieces. DMA engine assignment (measured): CONTIGUOUS
                # transfers <64KB round-robin 4-row batches across all 16
                # engines; strided (column-sliced) ones get a 2-engine path;
                # contiguous >=64KB pin to a single engine. Full-width
                # 18-row pieces (48.7KB) hit the fast path.
                hoist.append(sync.dma_start(out=wb_t[:], in_=wb_d[:]).then_inc(dsA, 16))
                for r0 in range(0, 88, 22):
                    hoist.append(
                        sync.dma_start(
                            out=w6_t[r0 : r0 + 22, :], in_=w6_d[r0 : r0 + 22, :]
                        ).then_inc(dsG, 16)
                    )
                # single full-width output store once both ACT copies land
                # (wait attached to the trigger: saves the standalone
                # EVENT_SEMAPHORE dispatch on the critical path)
                sync.dma_start(out=out_d[:, :], in_=out_t[:])._wait_ge(asem, 8).then_inc(dsO, 16)

            @block.vector
            def _(vector):
                # zb + psum_scr memsets gate only the ACT table preload
                hoist.append(vector.memset(psum_scr[:], 0.0).then_inc(ssev))
                hoist.append(vector.memset(zb_t[:], 0.0).then_inc(ssev))
                # (h, w) pooling as ONE 4-tap reduce over the (dh, dw) dims
                vector.wait_ge(ssem, 4)
                h2r = h2_t[:].rearrange(
                    "p (s hp dh wp dw) -> p (s hp) wp dh dw", s=NS, hp=3, dh=2, wp=3, dw=2
                )
                poolr = pool_t[0:6, :].rearrange("p (s hp wp) -> p (s hp) wp", s=NS, hp=3, wp=3)
                with nc.allow_low_precision("4-term bf16 pooling sum, matches prior impl"):
                    vector.tensor_reduce(
                        poolr[:], h2r[:], axis=mybir.AxisListType.XY, op=mybir.AluOpType.add
                    )._wait_ge(asem, 2).then_inc(vsem)  # 1


            @block.gpsimd
            def _(gpsimd):
                # ones memsets first (sig1 waits ssem at ~3.4us), then w4p
                # (needed ~5.5us) and the last w6 piece (needed ~7.4us).
                # Whole-tensor memsets: APs must start at partition 0;
                # writers of the non-ones rows wait on ssem first.
                hoist.append(gpsimd.memset(h1_t[:], 1.0).then_inc(ssem))
                hoist.append(gpsimd.memset(pool_t[:], 1.0).then_inc(ssem))
                hoist.append(gpsimd.memset(h4_t[:], 1.0).then_inc(ssem))
                hoist.append(gpsimd.memset(t01[:], 1.0).then_inc(ssem))
                hoist.append(gpsimd.dma_start(out=w4p_t[:], in_=w4p_d[:]).then_inc(dsE, 16))
                hoist.append(gpsimd.dma_start(out=w5t_t[:], in_=w5t_d[:]).then_inc(dsF, 16))
                hoist.append(
                    gpsimd.dma_start(out=w6_t[88:106, :], in_=w6_d[88:106, :]).then_inc(dsGs, 16)
                )

            @block.scalar
            def _(scalar):
                # x rides the ACT ring alone: the ACT engine starts ~260ns
                # before SP, so conv1's gate lands earliest here, and the
                # table-load dummy follows with no trigger in its way.
                hoist.append(scalar.dma_start(out=x_t[:], in_=x_d[:]).then_inc(dsA, 16))
                # dummy sigmoid FIRST IN THIS BASIC BLOCK: walrus tracks ACT
                # tables per-bb, so the preload must live in the same bb as
                # the real sigmoids to avoid a 1.3us reload before sig1.
                # Stage gates ride as attached waits (one per instruction).
                scalar.activation(scr_t[:], psum_scr[:], Sig, bias=zb_t[0:1, :])._wait_ge(ssev, 2)
                scalar.wait_ge(ssem, 4)
                scalar.activation(h1_t[0:24, :], psum1[:], Sig, bias=zb_t[0:24, :])._wait_ge(psem, 1).then_inc(asem)  # 1
                scalar.activation(h2_t[:], psum2[:], Sig, bias=zb_t[0:6, :])._wait_ge(psem, 2).then_inc(asem)  # 2
                scalar.activation(h4_t[0:80, :], psum4[:], Sig, bias=zb_t[0:80, :])._wait_ge(psem, 3).then_inc(asem)  # 3
                scalar.activation(
                    t01[0:100, 0:NS], psum5[:, 0:NS], Sig, bias=zb_t[0:100, :]
                )._wait_ge(psem, 4).then_inc(asem)  # 4
                scalar.activation(
                    t01[0:100, NS : 2 * NS], psum5[:, NS : 2 * NS], Sig, bias=zb_t[0:100, :]
                )._wait_ge(psem, 5).then_inc(asem)  # 5
                scalar.activation(h6_t[:], psum6[:], Sig, bias=zb_t[0:52, :])._wait_ge(psem, 6).then_inc(asem)  # 6
                scalar.copy(out_t[:, 0:168], psume[:, 0:168])._wait_ge(psem, 7).then_inc(asem)  # 7
                scalar.copy(out_t[:, 168:336], psume[:, 168:336])._wait_ge(psem, 8).then_inc(asem)  # 8

            @block.tensor
            def _(tensor):
                # conv1: 4 accumulated matmuls; K=53 incl. the b1 ones-row.
                # The wb+x gate rides as an attached wait on the first matmul.
                x4 = x_t[0:53, :].rearrange("p (s h w) -> p s h w", s=NS, h=7, w=24)
                taps1 = [(kh, kw) for kh in range(2) for kw in range(2)]
                for k, (kh, kw) in enumerate(taps1):
                    mm = tensor.matmul(
                        psum1[:],
                        wb_t[0:53, k * 24 : (k + 1) * 24],
                        x4[:, :, kh : kh + 6, kw : kw + 23 : 2],
                        start=(k == 0),
                        stop=(k == 3),
                    )
                    if k == 0:
                        mm._wait_ge(dsA, 2 * _DMA_CREDITS)  # wb (SP) + x (ACT)
                    if k == 3:
                        mm.then_inc(psem)  # psem 1
                # conv2: K=25 incl. the b2 ones-row. asem>=1 transitively
                # covers the Pool ones memsets (ACT waits ssem before sig1).
                h14 = h1_t[:].rearrange("p (s h w) -> p s h w", s=NS, h=6, w=12)
                for kw in range(2):
                    mm = tensor.matmul(
                        psum2[:],
                        wb_t[0:25, 96 + kw * 6 : 96 + (kw + 1) * 6],
                        h14[:, :, :, kw : kw + 11 : 2],
                        start=(kw == 0),
                        stop=(kw == 1),
                    )
                    if kw == 0:
                        mm._wait_ge(asem, 1)
                    if kw == 1:
                        mm.then_inc(psem)  # psem 2
                # early-satisfied DMA gate, placed off the hot handoffs
                tensor.wait_ge(dsE, 16)
                # fc4: 9 (hp,wp) matmuls vs the h/w-pooled tile; d-pooling and
                # /8 live in w4p; j=0 has K=7 incl. the b4 ones-row
                pool4 = pool_t[:].rearrange("p (s j) -> p s j", s=NS, j=9)
                for j in range(9):
                    kk = 7 if j == 0 else 6
                    mm = tensor.matmul(
                        psum4[:],
                        w4p_t[0:kk, j * 80 : (j + 1) * 80],
                        pool4[0:kk, :, j],
                        start=(j == 0),
                        stop=(j == 8),
                    )
                    if j == 0:
                        mm._wait_ge(vsem, 1)
                    if j == 8:
                        mm.then_inc(psem)  # psem 3
                # fc5: two k-halves, each gating its own sigmoid so fc6's
                # a-matmuls can start one ACT op earlier
                tensor.wait_ge(dsF, 16)
                tensor.matmul(
                    psum5[:, 0:NS], w5t_t[0:81, 0:100], h4_t[:], start=True, stop=True
                )._wait_ge(asem, 3).then_inc(psem)  # psem 4
                tensor.matmul(
                    psum5[:, NS : 2 * NS], w5t_t[0:81, 100:200], h4_t[:], start=True, stop=True
                ).then_inc(psem)  # psem 5
                # fc6: 13 i-chunks x 2 k-chunks (a_i, b_i interleaved — only
                # one PSUM accumulation group may be open per zero region)
                tensor.wait_ge(dsG, 64)
                tensor.wait_ge(dsGs, 16)
                for i in range(13):
                    mma = tensor.matmul(
                        psum6[:, i * NS : (i + 1) * NS],
                        w6_t[0:100, i * 52 : (i + 1) * 52],
                        t01[0:100, 0:NS],
                        start=True,
                        stop=False,
                    )
                    if i == 0:
                        mma._wait_ge(asem, 5)
                    mm = tensor.matmul(
                        psum6[:, i * NS : (i + 1) * NS],
                        w6_t[0:101, 676 + i * 52 : 676 + (i + 1) * 52],
                        t01[:, NS : 2 * NS],
                        start=False,
                        stop=True,
                    )
                    if i == 12:
                        mm.then_inc(psem)  # psem 6
                # einsum; each sample region releases its own copy engine
                h6v = h6_t[:].rearrange("p (i s) -> p s i", s=NS)
                for s in range(NS):
                    mm = tensor.matmul(
                        psume[:, s * 168 : (s + 1) * 168],
                        h6v[:, s, :],
                        x_t[0:52, s * 168 : (s + 1) * 168],
                        start=True,
                        stop=True,
                    ).then_inc(psem)  # psem 7, 8
                    if s == 0:
                        mm._wait_ge(asem, 6)

    _strip_entry_barrier(nc)
    return nc


def _strip_entry_barrier(nc):
    f = nc.m.functions[0]
    bbs = {bb.name: bb for bb in f.blocks}
    main = bbs["main"]
    # 1) drop the init all-engine barrier (nothing reads the const-AP tiles)
    main.instructions = [
        i
        for i in main.instructions
        if not (
            i.name.startswith("barrier_")
            or getattr(i, "opcode", "") == "Drain"
            or type(i).__name__ == "InstDrain"
        )
    ]
    # 2) hoist the input-DMA triggers into main so transfers start during the
    #    preamble, before the Block-entry rendezvous
    hoisted = {bi.ins.name for bi in getattr(nc, "_hoist_insts", [])}
    if hoisted:
        moved = []
        for bb in f.blocks:
            if bb.name == "main" or not bb.instructions:
                continue
            keep = []
            for i in bb.instructions:
                (moved if i.name in hoisted else keep).append(i)
            if len(keep) != len(bb.instructions):
                bb.instructions = keep
        # insert at the very top of main (after the entry Call): the DMA
        # triggers use only immediates + the parameter table, not the
        # preamble registers
        insts = main.instructions
        main.instructions = insts[:1] + moved + insts[1:]


def _prep_weights(w1, b1, w2, b2, w4, b4, w5, b5, w6, b6):
    f = np.float32
    w1v = np.asarray(w1, f)[0, 0]  # (6,2,2)
    w2v = np.asarray(w2, f)[0, 0, :, 0, :]  # (4,2)
    w4 = np.asarray(w4, f)
    w5 = np.asarray(w5, f)
    w6 = np.asarray(w6, f)
    b1 = np.asarray(b1, f)
    b2 = np.asarray(b2, f)
    b4 = np.asarray(b4, f)
    b5 = np.asarray(b5, f)
    b6 = np.asarray(b6, f)

    wb = np.zeros((64, 108), f)
    for kd in range(6):
        for kh in range(2):
            for kw in range(2):
                for d in range(24):
                    wb[2 * d + kd, (kh * 2 + kw) * 24 + d] = w1v[kd, kh, kw]
    wb[52, 0:24] = b1[0]  # ones-row bias, k=0 tap block only
    for kd in range(4):
        for kw in range(2):
            for d in range(6):
                wb[4 * d + kd, 96 + kw * 6 + d] = w2v[kd, kw]
    wb[24, 96:102] = b2[0]  # ones-row bias, kw=0 block only

    w4r = w4.reshape(80, 3, 3, 3) / 8.0
    w4q = np.transpose(w4r, (1, 2, 3, 0)).reshape(3, 720)
    w4p = np.zeros((12, 720), f)
    w4p[0:6:2, :] = w4q
    w4p[1:6:2, :] = w4q
    w4p[6, 0:80] = b4  # ones-row bias, j=0 block only

    w5t = np.zeros((86, 200), f)
    w5t[0:80, :] = w5.T
    w5t[80, :] = b5

    w6p = np.zeros((106, 1352), f)
    w6p[0:100, 0:676] = w6[:, 0:100].T
    w6p[0:100, 676:1352] = w6[:, 100:200].T
    w6p[100, 676:1352] = b6

    return dict(
        wb=wb.astype(BF16),
        w4p=w4p.astype(BF16),
        w5t=w5t.astype(BF16),
        w6=w6p.astype(BF16),
    )


def kernel(x, w1, b1, w2, b2, w4, b4, w5, b5, w6, b6, _trace=False):
    global LAST_EXEC_NS, LAST_RESULT
    from concourse.bass_utils import run_bass_kernel_spmd

    if "nc" not in _BUILT:
        _BUILT["nc"] = _build_nc()
    nc = _BUILT["nc"]

    xs = np.ascontiguousarray(np.asarray(x, np.float32).reshape(10, 52, 168))
    wd = _prep_weights(w1, b1, w2, b2, w4, b4, w5, b5, w6, b6)

    in_maps = []
    for i in range(N_CORES):
        xc = np.ones((64, NS * 168), np.float32)
        xc[0:52] = np.transpose(np.stack([xs[a] for a in ASSIGN[i]]), (1, 0, 2)).reshape(52, NS * 168)
        xc = np.ascontiguousarray(xc.astype(BF16))
        m = {"x": xc}
        m.update(wd)
        in_maps.append(m)

    res = run_bass_kernel_spmd(nc, in_maps, core_ids=list(range(N_CORES)), trace=_trace)
    LAST_EXEC_NS = res.exec_time_ns
    LAST_RESULT = res

    out = np.zeros((10, 2184), np.float32)
    for i in range(N_CORES):
        o = res.results[i]["out"].reshape(13, NS, 168)
        for slot, b in enumerate(ASSIGN[i]):
            out[b] = o[:, slot, :].reshape(2184)
    return out
